# revision 37
# baseline (speedup 1.0000x reference)
"""Trainium2 Bass kernel for nn_MeanShift (retrieval_knn).

Full-input contract: kernel(**inputs) -> (loss, purity).

Shipped design (MODE="v7"; earlier modes kept for reference):
  - Bank (K=128000) sharded across 8 cores (16000 rows each); targets
    replicated.  Host casts inputs to fp8e4 with power-of-2 scales
    (bank x8, normalized t x64) and packs a per-partition-contiguous
    layout per 1000-row super-tile.
  - Per core, 16 super-tiles: per-super 524KB DMAs (even: sync HWDGE
    ring, odd: GpSimd SWDGE ring, triggers prefetched 3 ahead), fp8
    DoubleRow matmuls (2x128 contraction planes, 0.5 cyc/row -- measured
    at the 157 TF/s fp8 peak, one 500-col MM issued per 213ns), PSUM
    [128, 2, 512] tiles.
  - Chunk scoring reads PSUM directly, split across two engines per
    super: half h=0 -> ScalarE Exp activation in-place with accum_out
    (log-sum-exp score, s_eff=256), half h=1 -> DVE max8.  One packed
    [128, 16, 9] output tile, single DMA.
  - Host epilogue: per-row proxies over all 128 chunk scores (ln(score)/
    256 or cand0/512 ~ chunk max in cosine units), top-16 1000-wide
    windows recomputed exactly in fp32, global top-5 with reference
    tie-breaking (lowest index), then dist_q/loss/purity.

Selection correctness: on the fixed inputs the worst true-top-5 chunk
ranks 10th of 128 by either proxy (fp8 sim noise sigma=1.7e-3), so W=16
windows cover every row with margin; the exact-fp32 window recompute
then reproduces the reference answer bit-for-bit (rel err 0.0 on HW).

DMA ring split (v14-v17 tuning): the GpSimd SWDGE ring sustains only
~115GB/s vs the sync HWDGE ring's ~180-230GB/s warm, so supers split
11 (sync) / 5 (gpsimd, every 3rd from su2).  su0 AND su1 are halved
across the sync + ACT HWDGE rings (the ACT ring is free until ScalarE's
first consumer), and 14 dummy DoubleRow matmuls on a zeroed tile bridge
the ~3us DMA ramp so the HAM clock gate is open when su0's real
matmuls issue (start-only: pad blocks BETWEEN supers stall on
scheduler semaphores).

Measured per-core NEFF exec (neuron-profile): 47.5-50.3us; engine
floors: PE 27.3us (fp8 peak), DMA ~24us, ACT ~23us, DVE ~19us, plus
~3us ramp and ~10us fixed BIR exit barrier.
"""

import numpy as np
import ml_dtypes

import jax
from jax.experimental.shard_map import shard_map
from jax.sharding import Mesh, PartitionSpec

import concourse.bass as bass
import concourse.bacc as bacc
import concourse.mybir as mybir
import concourse.tile as tile
from concourse import bass2jax

N_CORES = 8
B = 256          # batch (rows of query/current_target)
C = 512          # feature dim
K = 128000       # memory bank size
KL = K // N_CORES  # 16000 bank rows per core
KT = 500         # matmul k-tile width (PSUM bank holds 512 fp32)
GRP = 4          # k-tiles per max-scan chunk (v2 path)
CHUNK = KT * GRP   # 2000 elements per DVE max8 scan (v2 path)
N_GRP = KL // CHUNK  # 8 scan chunks per core (v2 path)
NCAND = 8 * N_GRP    # 64 candidates per row per core (v2 path)
TOPK = 5
EPS = 1e-12


def groups_for(kl):
    """v1 scan-chunk widths. Six 500-wide leading groups cut the DVE
    start-up ramp; 1000-wide steady-state chunks schedule tighter than
    2000 (TimelineSim: 84.5us vs 87.8us per core for kl=16000)."""
    if kl >= 4000 and (kl - 3000) % 1000 == 0:
        return [500] * 6 + [1000] * ((kl - 3000) // 1000)
    assert kl % KT == 0
    return [KT] * (kl // KT)

# bfloat16 halves DMA + PE time; fp32 is the accuracy-safe fallback.
# Validated on the fixed inputs: bf16 changes 15/256 rows' top-5 with min
# 5th/6th sim gap 2.9e-4 (>> HW accumulation noise), loss rel err 4.8e-5,
# purity identical (0.0) -- well inside the 2e-2 gate.
DTYPE = mybir.dt.bfloat16

# v2 (tagged single-scan) constants. Device computes sims scaled to
# |sim| <= 0.25 (host passes t_norm/4; actual |sim| ~ 0.05). Per 500-wide
# matmul tile the PE appends three rank-1 accumulations, in order:
#   +4.0   -- rounds sim onto the 2^-21 grid (exponent pinned at 2^2)
#   -4.0   -- Sterbenz-exact unshift, psum = q(sim), a 2^-21 multiple
#   +id*2^-25, id in [0,16) the 125-wide subchunk of the column -- exact
#          (ulp <= 2^-26 for |q| < 0.25), and SUB-quantum, so packed
#          ordering matches q(sim) ordering to within one quantum.
# One max8 scan returns packed = q(sim) + id*2^-25; the host decodes
# id = (packed/2^-25) mod 16 (q/2^-25 is a multiple of 16 for the
# positive sims that matter) and re-derives exact values by recomputing
# the winners' 125-wide windows.
N_SUB_PER_KT = 4          # 4 subchunks of 125 per 500-wide k-tile
SUB = KT // N_SUB_PER_KT  # 125
N_SUB = CHUNK // SUB      # 16 subchunk ids per 2000-wide scan chunk
TAG_EPS = 2.0 ** -25
QCONST = 4.0
SIM_SCALE = 0.25          # host scales t_norm by this before casting

LAST_RESULTS = None    # per-core output dicts of the most recent run


def build_nc(dtype=DTYPE, kl=KL, with_index=True):
    """Build the single-core Bass program (SPMD across 8 cores).

    with_index=False (v3): drop the max_index pass and cand_i output --
    the host recovers indices by recomputing the <=8 winning 500-wide
    chunks per row (candidate slot -> chunk is static). Halves DVE work.
    """
    groups = [KT] * (kl // KT) if not with_index else groups_for(kl)
    n_grp = len(groups)
    ncand = 8 * n_grp
    mx = max(groups)
    # Bacc (not raw Bass): its compile() passes split multi-semaphore waits
    # (move_matmul_waits_to_ldweights / generate_event_semaphores) that the
    # walrus codegen's 1-wait-per-instruction limit requires.
    nc = bacc.Bacc()
    bankT = nc.declare_dram_parameter("bankT", [C, kl], dtype, isOutput=False)
    tT = nc.declare_dram_parameter("tT", [C, B], dtype, isOutput=False)
    cand_v = nc.declare_dram_parameter(
        "cand_v", [B, ncand], mybir.dt.float32, isOutput=True
    )
    cand_i = None
    if with_index:
        cand_i = nc.declare_dram_parameter(
            "cand_i", [B, ncand], mybir.dt.uint32, isOutput=True
        )

    bankT_r = bankT.rearrange("(c p) k -> p c k", p=128)  # [128, 4, kl]
    tT_r = tT.rearrange("(c p) b -> p c b", p=128)        # [128, 4, B]

    with tile.TileContext(nc) as tc:
        with (
            tc.tile_pool(name="const", bufs=1) as constp,
            # bufs=4: with the max_index pass gone the PE chain paces the
            # schedule, and 4-deep bank prefetch keeps it fed (model:
            # 67.5us vs 70.5us at bufs=3; saturates at 4).
            tc.tile_pool(name="bank", bufs=4) as bankp,
            tc.tile_pool(name="sim", bufs=2) as simp,
            tc.tile_pool(name="cand", bufs=1) as candp,
            tc.tile_pool(name="ps", bufs=8, space="PSUM") as psp,
        ):
            tw = constp.tile([128, 4, B], dtype)
            nc.sync.dma_start(tw[:], tT_r[:])

            vals = [
                candp.tile([128, n_grp, 8], mybir.dt.float32, tag=f"v{b}", name=f"vals{b}")
                for b in range(2)
            ]
            idxs = None
            if with_index:
                idxs = [
                    candp.tile([128, n_grp, 8], mybir.dt.uint32, tag=f"i{b}", name=f"idxs{b}")
                    for b in range(2)
                ]

            kt = 0
            for g, chunk in enumerate(groups):
                sims = [
                    simp.tile([128, mx], mybir.dt.float32, tag=f"s{b}", name=f"sim{b}")
                    for b in range(2)
                ]
                for j in range(chunk // KT):
                    bk = bankp.tile([128, 4, KT], dtype, tag="bank")
                    if kt == 0:
                        # split the first load per c-chunk so the first
                        # matmul starts after 1/4 of the transfer
                        # (model: 64.7us vs 67.5us)
                        for c in range(4):
                            nc.sync.dma_start(
                                bk[:, c, :], bankT_r[:, c, 0:KT]
                            )
                    else:
                        nc.sync.dma_start(
                            bk[:], bankT_r[:, :, kt * KT:(kt + 1) * KT]
                        )
                    for b in range(2):
                        ps = psp.tile([128, KT], mybir.dt.float32, tag="ps")
                        for c in range(4):
                            nc.tensor.matmul(
                                ps[:],
                                tw[:, c, b * 128:(b + 1) * 128],
                                bk[:, c, :],
                                start=(c == 0),
                                stop=(c == 3),
                            )
                        nc.scalar.copy(sims[b][:, j * KT:(j + 1) * KT], ps[:])
                    kt += 1
                for b in range(2):
                    nc.vector.max(vals[b][:, g, :], sims[b][:, 0:chunk])
                    if with_index:
                        nc.vector.max_index(
                            idxs[b][:, g, :], vals[b][:, g, :], sims[b][:, 0:chunk]
                        )

            for b in range(2):
                nc.sync.dma_start(cand_v[b * 128:(b + 1) * 128, :], vals[b][:])
                if with_index:
                    nc.sync.dma_start(cand_i[b * 128:(b + 1) * 128, :], idxs[b][:])

    return nc


def _make_consts():
    """Host-side constant rows for the v2 tag matmuls, bf16 [1, 3500].

    Layout: [0:128) ones (rank-1 stationary); [500:1000) +4.0;
    [1000:1500) -4.0; [1500+j*500 : 2000+j*500) tag row for kt%4 == j:
    id*2^-25 with id = ((j*500+n) // SUB) % 16. All exact in bf16.
    """
    c = np.zeros((1, 3500), np.float32)
    c[0, 0:128] = 1.0
    c[0, 500:1000] = QCONST
    c[0, 1000:1500] = -QCONST
    n = np.arange(KT)
    for j in range(4):
        ids = (j * KT + n) // SUB % N_SUB
        c[0, 1500 + j * 500:2000 + j * 500] = ids * TAG_EPS
    return c.astype(ml_dtypes.bfloat16)


def build_nc_v2(dtype=mybir.dt.bfloat16, kl=KL):
    """Tagged single-scan variant: one DVE max8 pass, no max_index."""
    assert dtype == mybir.dt.bfloat16
    n_grp = kl // CHUNK
    ncand = 8 * n_grp
    nc = bacc.Bacc()
    bankT = nc.declare_dram_parameter("bankT", [C, kl], dtype, isOutput=False)
    tT = nc.declare_dram_parameter("tT", [C, B], dtype, isOutput=False)
    consts = nc.declare_dram_parameter("consts", [1, 3500], dtype, isOutput=False)
    cand_v = nc.declare_dram_parameter(
        "cand_v", [B, ncand], mybir.dt.float32, isOutput=True
    )

    bankT_r = bankT.rearrange("(c p) k -> p c k", p=128)  # [128, 4, kl]
    tT_r = tT.rearrange("(c p) b -> p c b", p=128)        # [128, 4, B]

    with tile.TileContext(nc) as tc:
        with (
            tc.tile_pool(name="const", bufs=1) as constp,
            tc.tile_pool(name="bank", bufs=3) as bankp,
            tc.tile_pool(name="sim", bufs=2) as simp,
            tc.tile_pool(name="cand", bufs=1) as candp,
            tc.tile_pool(name="ps", bufs=8, space="PSUM") as psp,
        ):
            tw = constp.tile([128, 4, B], dtype)
            nc.sync.dma_start(tw[:], tT_r[:])
            cst = constp.tile([1, 3500], dtype)
            nc.sync.dma_start(cst[:], consts[:])
            ones_r = cst[0:1, 0:128]
            q_r = cst[0:1, 500:1000]
            nq_r = cst[0:1, 1000:1500]
            tag_r = [cst[0:1, 1500 + j * 500:2000 + j * 500] for j in range(4)]

            vals = [
                candp.tile([128, n_grp, 8], mybir.dt.float32,
                           tag=f"v{b}", name=f"vals{b}")
                for b in range(2)
            ]

            for g in range(n_grp):
                sims = [
                    simp.tile([128, CHUNK], mybir.dt.float32,
                              tag=f"s{b}", name=f"sim{b}")
                    for b in range(2)
                ]
                for j in range(GRP):
                    kt = g * GRP + j
                    bk = bankp.tile([128, 4, KT], dtype, tag="bank")
                    nc.sync.dma_start(
                        bk[:], bankT_r[:, :, kt * KT:(kt + 1) * KT]
                    )
                    for b in range(2):
                        ps = psp.tile([128, KT], mybir.dt.float32, tag="ps",
                                      name="ps")
                        for c in range(4):
                            nc.tensor.matmul(
                                ps[:],
                                tw[:, c, b * 128:(b + 1) * 128],
                                bk[:, c, :],
                                start=(c == 0), stop=False,
                            )
                        # quantize then tag: +4, -4, +id*2^-25 (in order)
                        nc.tensor.matmul(ps[:], ones_r, q_r,
                                         start=False, stop=False)
                        nc.tensor.matmul(ps[:], ones_r, nq_r,
                                         start=False, stop=False)
                        nc.tensor.matmul(ps[:], ones_r, tag_r[j % 4],
                                         start=False, stop=True)
                        nc.scalar.copy(sims[b][:, j * KT:(j + 1) * KT], ps[:])
                for b in range(2):
                    nc.vector.max(vals[b][:, g, :], sims[b][:])

            for b in range(2):
                nc.sync.dma_start(cand_v[b * 128:(b + 1) * 128, :], vals[b][:])

    return nc


# ---------------------------------------------------------------------------
# v4: fp8 DoubleRow + two-engine chunk scoring.
#
#   - Inputs cast to fp8e4 host-side with power-of-2 scales (bank x8,
#     normalized t x64): halves DMA bytes vs bf16 (16.4 -> 8.2 MB/core) and
#     the PE runs DoubleRow fp8 (2 contraction planes of 128 per matmul,
#     0.5 cyc/row): 2 matmuls per 500-wide tile per 128-row half.
#   - Chunk scoring splits across two engines reading PSUM directly (the
#     old ScalarE-evict + DVE-max8 pipeline cost ~46us on EACH engine):
#       * 13/32 tiles: DVE max8 straight from PSUM -> top-8 values.
#       * 19/32 tiles: ScalarE Exp activation in-place in PSUM with
#         accum_out -> sum(exp(0.5*psum)) = a log-sum-exp chunk score.
#     Both reduce to a per-(row, 500-chunk) proxy for the chunk max
#     (psum = 512*sim_n, so LSE exponent scale = 256; ln(score)/256 ~ max
#     ~ cand0/512), comparable across engines on the host.
#   - Host epilogue: rank all 256 chunk proxies per row, exactly recompute
#     the top-W windows in fp32 (device is only a candidate generator) and
#     take the global top-5 with reference tie-breaking.
#
# Selection margin validated offline on the fixed inputs: fp8 sim noise
# sigma=1.7e-3; worst true-top-5 chunk rank under either proxy = 10 (of
# 256), so W=16 windows cover all 256 rows with 6 ranks of margin.
# ---------------------------------------------------------------------------
V4_T = KL // KT            # 32 tiles of 500 bank rows per core
V4_S_BANK = 8.0            # bank fp8 scale (power of 2; avoids subnormals)
V4_S_T = 64.0              # normalized-t fp8 scale
V4_ACT_SCALE = 0.5         # Exp scale on psum; s_eff = 8*64*0.5 = 256
V4_SEFF = V4_S_BANK * V4_S_T * V4_ACT_SCALE
V4_N_DVE = 13              # tiles scored by DVE max8 (rest: ScalarE LSE)
V4_W = 16                  # host-recomputed candidate windows per row


def v4_dve_tiles():
    """13 DVE-scored tiles spread evenly over the 32 (Bresenham)."""
    return [t for t in range(V4_T)
            if (t + 1) * V4_N_DVE // V4_T > t * V4_N_DVE // V4_T]


def build_nc_v4():
    """fp8 DoubleRow + split ACT/DVE chunk scoring (see module comment)."""
    fp8 = mybir.dt.float8e4
    dve_tiles = set(v4_dve_tiles())
    n_dve = len(dve_tiles)
    n_act = V4_T - n_dve
    nc = bacc.Bacc()
    # [p, t, j, i, n] = bank8[t*500+n, j*256+i*128+p]: per partition each
    # tile's 2000 bytes are contiguous (2KB DMA runs, vs 1KB strided in v3)
    bank8 = nc.declare_dram_parameter(
        "bank8", [128, V4_T, 2, 2, KT], fp8, isOutput=False
    )
    # [p, h, j, i, m] = t8[h*128+m, j*256+i*128+p]
    tw8 = nc.declare_dram_parameter("tw8", [128, 2, 2, 2, 128], fp8,
                                    isOutput=False)
    cand_v = nc.declare_dram_parameter(
        "cand_v", [B, n_dve, 8], mybir.dt.float32, isOutput=True
    )
    scores = nc.declare_dram_parameter(
        "scores", [B, n_act], mybir.dt.float32, isOutput=True
    )

    with tile.TileContext(nc) as tc:
        with (
            tc.tile_pool(name="const", bufs=1) as constp,
            tc.tile_pool(name="bank", bufs=4) as bankp,
            tc.tile_pool(name="cand", bufs=1) as candp,
            tc.tile_pool(name="ps", bufs=8, space="PSUM") as psp,
        ):
            tw = constp.tile([128, 2, 2, 2, 128], fp8)
            nc.sync.dma_start(tw[:], tw8[:])

            cands = [
                candp.tile([128, n_dve, 8], mybir.dt.float32,
                           tag=f"c{h}", name=f"cands{h}")
                for h in range(2)
            ]
            scs = [
                candp.tile([128, n_act], mybir.dt.float32,
                           tag=f"s{h}", name=f"scores{h}")
                for h in range(2)
            ]

            d_idx = a_idx = 0
            for t in range(V4_T):
                bk = bankp.tile([128, 2, 2, KT], fp8, tag="bank")
                if t == 0:
                    # split the first load so the first matmul starts after
                    # a quarter of the transfer
                    for j in range(2):
                        for i in range(2):
                            nc.sync.dma_start(bk[:, j, i], bank8[:, 0, j, i])
                else:
                    nc.sync.dma_start(bk[:], bank8[:, t])
                for h in range(2):
                    ps = psp.tile([128, KT], mybir.dt.float32, tag="ps")
                    for j in range(2):
                        nc.tensor.matmul(
                            ps[:],
                            tw[:, h, j],        # [128, 2, 128] stationary
                            bk[:, j],           # [128, 2, 500] moving
                            start=(j == 0),
                            stop=(j == 1),
                            perf_mode=mybir.MatmulPerfMode.DoubleRow,
                        )
                    if t in dve_tiles:
                        nc.vector.max(cands[h][:, d_idx, :], ps[:])
                    else:
                        nc.scalar.activation(
                            ps[:], ps[:],
                            mybir.ActivationFunctionType.Exp,
                            scale=V4_ACT_SCALE,
                            accum_out=scs[h][:, a_idx:a_idx + 1],
                        )
                if t in dve_tiles:
                    d_idx += 1
                else:
                    a_idx += 1

            for h in range(2):
                nc.sync.dma_start(cand_v[h * 128:(h + 1) * 128, :], cands[h][:])
                nc.sync.dma_start(scores[h * 128:(h + 1) * 128, :], scs[h][:])

    return nc


def v4_pack_inputs(t, bank):
    """Host-side fp8 packing for v4. Returns (bank8 [8*128, ...], tw8)."""
    f8 = ml_dtypes.float8_e4m3
    t_n = t / np.maximum(np.linalg.norm(t, axis=1, keepdims=True), EPS)
    t8 = (t_n * V4_S_T).astype(f8)                      # [B, C]
    b8 = (bank * V4_S_BANK).astype(f8)                  # [K, C]
    # bank8[p, t, j, i, n] = b8[core*KL + t*KT + n, j*256 + i*128 + p]
    bank8 = (
        b8.reshape(N_CORES, V4_T, KT, 2, 2, 128)        # m, t, n, j, i, p
        .transpose(0, 5, 1, 3, 4, 2)                    # m, p, t, j, i, n
        .reshape(N_CORES * 128, V4_T, 2, 2, KT)
    )
    # tw8[p, h, j, i, m] = t8[h*128+m, j*256+i*128+p]
    tw8_1 = (
        t8.reshape(2, 128, 2, 2, 128)                   # h, m, j, i, p
        .transpose(4, 0, 2, 3, 1)                       # p, h, j, i, m
    )
    tw8 = np.concatenate([tw8_1] * N_CORES, axis=0)
    return np.ascontiguousarray(bank8), np.ascontiguousarray(tw8)


def _run_v4(exe, t, bank):
    """fp8 candidate-generator path: returns per-row global top-5 indices."""
    global LAST_RESULTS
    bank8, tw8 = v4_pack_inputs(t, bank)
    concat = {"bank8": bank8, "tw8": tw8}
    results = exe([concat[n] for n in exe.in_names])
    LAST_RESULTS = results

    dve_tiles = v4_dve_tiles()
    act_tiles = [t_ for t_ in range(V4_T) if t_ not in set(dve_tiles)]
    # per-chunk proxy for the chunk max, in normalized-sim units
    proxy = np.empty((B, N_CORES, V4_T), np.float32)
    for m, r in enumerate(results):
        proxy[:, m, dve_tiles] = r["cand_v"][:, :, 0] / (V4_S_BANK * V4_S_T)
        proxy[:, m, act_tiles] = np.log(
            np.maximum(r["scores"], 1e-30)) / V4_SEFF
    proxy = proxy.reshape(B, N_CORES * V4_T)

    sel = np.argpartition(-proxy, V4_W - 1, axis=1)[:, :V4_W]  # [B, W] chunks
    t_n = t / np.maximum(np.linalg.norm(t, axis=1, keepdims=True), EPS)
    span = np.arange(KT, dtype=np.int64)
    top5 = np.empty((B, TOPK), np.int64)
    for b in range(B):
        widx = (sel[b].astype(np.int64)[:, None] * KT + span[None, :]).ravel()
        wsims = bank[widx] @ t_n[b]                     # fp32 exact windows
        o = np.lexsort((widx, -wsims))
        top5[b] = widx[o[:TOPK]]
    return top5


# ---------------------------------------------------------------------------
# v5: v4 + wider units and DMA/LDW batching, from the v4 HW trace:
#   - DMA active was 41-47us for 8.45MB (32 per-tile DMAs serialized on one
#     HWDGE ring, ~0.6us fixed each).  v5 loads 1MB groups (8 DMAs) and
#     alternates the sync/scalar HWDGE rings so fixed costs overlap.
#   - PE active was 36.4us (128 LDWEIGHTS, one per matmul -- DoubleRow
#     disables fast-weight-load).  v5 sweeps each stationary across a group
#     of 2 super-tiles (4 matmuls back-to-back per LDW).
#   - ACT 33.8us vs DVE 19us was unbalanced, and per-instruction overhead
#     (~400 cyc) dominated 500-elem scans.  v5 scores 1000-wide units
#     ([128, 2, 500] strided PSUM APs over a 2-bank tile) with an 8/8
#     super-tile split: ~20us each.
# Chunk width 1000: worst true-top-5 chunk rank on the fixed inputs = 10 of
# 128 (either proxy), W=16 windows -> margin 6.
# ---------------------------------------------------------------------------
V5_CW = 1000               # scoring chunk width (super-tile)
V5_T = KL // V5_CW         # 16 super-tiles per core
V5_G = 2                   # super-tiles per stationary-sweep group
V5_NG = V5_T // V5_G       # 8 groups (1MB DMA each)
V5_W = 16                  # host-recomputed 1000-wide windows per row


def v5_act_supers():
    """8 of 16 super-tiles scored by ScalarE LSE (alternating)."""
    return [s for s in range(V5_T) if s % 2 == 0]


def build_nc_v5():
    fp8 = mybir.dt.float8e4
    act_set = set(v5_act_supers())
    n_act = len(act_set)
    n_dve = V5_T - n_act
    nc = bacc.Bacc()
    # [p, g, s, j, i, n] = b8[(g*2+s)*1000 + n, j*256 + i*128 + p]
    bank8 = nc.declare_dram_parameter(
        "bank8", [128, V5_NG, V5_G, 2, 2, V5_CW], fp8, isOutput=False
    )
    tw8 = nc.declare_dram_parameter("tw8", [128, 2, 2, 2, 128], fp8,
                                    isOutput=False)
    cand_v = nc.declare_dram_parameter(
        "cand_v", [B, n_dve, 8], mybir.dt.float32, isOutput=True
    )
    scores = nc.declare_dram_parameter(
        "scores", [B, n_act], mybir.dt.float32, isOutput=True
    )

    with tile.TileContext(nc) as tc:
        with (
            tc.tile_pool(name="const", bufs=1) as constp,
            tc.tile_pool(name="bank", bufs=3) as bankp,
            tc.tile_pool(name="cand", bufs=1) as candp,
            tc.tile_pool(name="ps", bufs=4, space="PSUM") as psp,
        ):
            tw = constp.tile([128, 2, 2, 2, 128], fp8)
            nc.sync.dma_start(tw[:], tw8[:])

            cands = [
                candp.tile([128, n_dve, 8], mybir.dt.float32,
                           tag=f"c{h}", name=f"cands{h}")
                for h in range(2)
            ]
            scs = [
                candp.tile([128, n_act], mybir.dt.float32,
                           tag=f"s{h}", name=f"scores{h}")
                for h in range(2)
            ]

            d_idx = a_idx = 0
            for g in range(V5_NG):
                bk = bankp.tile([128, V5_G, 2, 2, V5_CW], fp8, tag="bank")
                dma_eng = nc.sync if g % 2 == 0 else nc.scalar
                if g == 0:
                    # finer first loads so the first matmuls start early
                    for s in range(V5_G):
                        for j in range(2):
                            nc.sync.dma_start(bk[:, s, j], bank8[:, 0, s, j])
                else:
                    dma_eng.dma_start(bk[:], bank8[:, g])
                # 2-bank psum tile per (h, s); matmul writes 500-col halves
                pss = [[psp.tile([128, 2, 512], mybir.dt.float32, tag="ps",
                                 name="ps")
                        for s in range(V5_G)] for h in range(2)]
                # stationary-major sweep: one LDW serves 4 matmuls
                for j in range(2):
                    for h in range(2):
                        for s in range(V5_G):
                            for c in range(2):
                                nc.tensor.matmul(
                                    pss[h][s][:, c, 0:500],
                                    tw[:, h, j],
                                    bk[:, s, j, :, c * 500:(c + 1) * 500],
                                    start=(j == 0),
                                    stop=(j == 1),
                                    perf_mode=mybir.MatmulPerfMode.DoubleRow,
                                )
                for s in range(V5_G):
                    su = g * V5_G + s
                    for h in range(2):
                        view = pss[h][s][:, :, 0:500]
                        if su in act_set:
                            nc.scalar.activation(
                                view, view,
                                mybir.ActivationFunctionType.Exp,
                                scale=V4_ACT_SCALE,
                                accum_out=scs[h][:, a_idx:a_idx + 1],
                            )
                        else:
                            nc.vector.max(cands[h][:, d_idx, :], view)
                    if su in act_set:
                        a_idx += 1
                    else:
                        d_idx += 1

            for h in range(2):
                nc.sync.dma_start(cand_v[h * 128:(h + 1) * 128, :], cands[h][:])
                nc.sync.dma_start(scores[h * 128:(h + 1) * 128, :], scs[h][:])

    return nc


def v5_pack_inputs(t, bank):
    """Host-side fp8 packing for v5."""
    f8 = ml_dtypes.float8_e4m3
    t_n = t / np.maximum(np.linalg.norm(t, axis=1, keepdims=True), EPS)
    t8 = (t_n * V4_S_T).astype(f8)
    b8 = (bank * V4_S_BANK).astype(f8)
    bank8 = (
        b8.reshape(N_CORES, V5_NG, V5_G, V5_CW, 2, 2, 128)  # m,g,s,n,j,i,p
        .transpose(0, 6, 1, 2, 4, 5, 3)                     # m,p,g,s,j,i,n
        .reshape(N_CORES * 128, V5_NG, V5_G, 2, 2, V5_CW)
    )
    tw8_1 = (
        t8.reshape(2, 128, 2, 2, 128)                       # h,m,j,i,p
        .transpose(4, 0, 2, 3, 1)                           # p,h,j,i,m
    )
    tw8 = np.concatenate([tw8_1] * N_CORES, axis=0)
    return np.ascontiguousarray(bank8), np.ascontiguousarray(tw8)


def _run_v5(exe, t, bank):
    global LAST_RESULTS
    bank8, tw8 = v5_pack_inputs(t, bank)
    concat = {"bank8": bank8, "tw8": tw8}
    results = exe([concat[n] for n in exe.in_names])
    LAST_RESULTS = results
    return _v5_select(results, t, bank)


# ---------------------------------------------------------------------------
# v10: v9 + three-ring cold start.  Cold DMA rings sustain only ~100GB/s
# for the first ~10us (~1 super per ring per 5us), but the PE consumes a
# super per 1.71us.  v10 spreads the first 8 supers across sync, GpSimd
# AND the ACT ring (ACT's own work only starts once su0 is computed), with
# 6-deep prefetch, and sizes the HAM warm-up to the su0 latency.
# ---------------------------------------------------------------------------
V10_PREFETCH = 6
V10_WARM_MM = 18
V10_PAD_MM = 6


def v10_ring(su):
    """Ring per super, ordered by expected COLD arrival time: rings run
    ~100GB/s for their first ~10us, so each ring's k-th transfer lands at
    ~(k * 5us); program order must match arrival order or the PE stalls
    on a super whose ring is still busy."""
    # weighted by measured ring rates (sync/scalar HWDGE ~200GB/s,
    # GpSimd SWDGE ~115GB/s): period-4 pattern sync,gpsimd,scalar,sync
    early = {1: "sync", 2: "gpsimd", 3: "scalar"}
    if su in early:
        return early[su]
    return ("sync", "gpsimd", "scalar", "sync")[su % 4]


def build_nc_v10():
    fp8 = mybir.dt.float8e4
    nc = bacc.Bacc()
    bank8 = nc.declare_dram_parameter(
        "bank8", [128, V5_T, 2, 2, V5_CW], fp8, isOutput=False
    )
    tw8 = nc.declare_dram_parameter("tw8", [128, 2, 2, 2, 128], fp8,
                                    isOutput=False)
    outs_d = nc.declare_dram_parameter(
        "outs", [128, V5_T, 9], mybir.dt.float32, isOutput=True
    )

    with tile.TileContext(nc) as tc:
        with (
            tc.tile_pool(name="const", bufs=1) as constp,
            tc.tile_pool(name="bank", bufs=V10_PREFETCH + 1) as bankp,
            tc.tile_pool(name="cand", bufs=1) as candp,
            tc.tile_pool(name="ps", bufs=4, space="PSUM") as psp,
        ):
            tw = constp.tile([128, 2, 2, 2, 128], fp8)
            warm = constp.tile([128, 2, 256], fp8)
            nc.vector.memset(warm[:], 0.0)

            outs = candp.tile([128, V5_T, 9], mybir.dt.float32)

            bks = {}
            engs = {"sync": nc.sync, "gpsimd": nc.gpsimd, "scalar": nc.scalar}

            def issue_dma(su):
                bk = bankp.tile([128, 2, 2, V5_CW], fp8, tag="bank",
                                name="bk")
                if su == 0:
                    # split the cold first super across both HWDGE rings
                    nc.sync.dma_start(bk[:, 0], bank8[:, 0, 0])
                    nc.scalar.dma_start(tw[:], tw8[:])
                    nc.scalar.dma_start(bk[:, 1], bank8[:, 0, 1])
                else:
                    engs[v10_ring(su)].dma_start(bk[:], bank8[:, su])
                bks[su] = bk

            for su in range(V10_PREFETCH):
                issue_dma(su)

            wps = psp.tile([128, 2, 512], mybir.dt.float32, tag="ps",
                           name="wps")
            for _ in range(V10_WARM_MM):
                nc.tensor.matmul(
                    wps[:, 0, 0:256],
                    warm[:, :, 0:128],
                    warm[:],
                    start=True, stop=True,
                    perf_mode=mybir.MatmulPerfMode.DoubleRow,
                )

            for su in range(V5_T):
                if su + V10_PREFETCH < V5_T:
                    issue_dma(su + V10_PREFETCH)
                if su == 1:
                    # bridge the su0->su1 cold-arrival gap (~1.5us) so the
                    # HAM clock gate stays open
                    pps = psp.tile([128, 2, 512], mybir.dt.float32,
                                   tag="ps", name="pps")
                    for _ in range(V10_PAD_MM):
                        nc.tensor.matmul(
                            pps[:, 0, 0:256],
                            warm[:, :, 0:128],
                            warm[:],
                            start=True, stop=True,
                            perf_mode=mybir.MatmulPerfMode.DoubleRow,
                        )
                bk = bks.pop(su)
                pss = [psp.tile([128, 2, 512], mybir.dt.float32, tag="ps",
                                name="ps")
                       for h in range(2)]
                for h in range(2):
                    for j in range(2):
                        for c in range(2):
                            nc.tensor.matmul(
                                pss[h][:, c, 0:500],
                                tw[:, h, j],
                                bk[:, j, :, c * 500:(c + 1) * 500],
                                start=(j == 0),
                                stop=(j == 1),
                                perf_mode=mybir.MatmulPerfMode.DoubleRow,
                            )
                nc.scalar.activation(
                    pss[0][:, :, 0:500], pss[0][:, :, 0:500],
                    mybir.ActivationFunctionType.Exp,
                    scale=V4_ACT_SCALE,
                    accum_out=outs[:, su, 8:9],
                )
                nc.vector.max(outs[:, su, 0:8], pss[1][:, :, 0:500])

            nc.sync.dma_start(outs_d[:], outs[:])

    return nc


def _run_v10(exe, t, bank):
    global LAST_RESULTS
    bank8, tw8 = v6_pack_inputs(t, bank)
    concat = {"bank8": bank8, "tw8": tw8}
    results = exe([concat[n] for n in exe.in_names])
    LAST_RESULTS = results
    return _v7_select(results, t, bank)


# ---------------------------------------------------------------------------
# v9: v7 + HAM warm-up + split cold-start, from the v7/v8 HW traces:
#   - v7's remaining MM-phase excess (~5us over the 27.3us fp8 peak) was
#     HAM clock-gate throttle: the PE sits idle through the ~6us DMA ramp,
#     so the first ~40 matmuls issue at half clock.  v9 issues 24 dummy
#     DoubleRow matmuls on a zeroed SBUF tile during the ramp -- free
#     (PE idle anyway) -- so real matmuls start at full clock.
#   - cold-start split as v7b: su0's two j-planes on the sync + ACT rings
#     (tw first, 131KB), su1 on GpSimd -> first MM ~3us earlier than v7.
# ---------------------------------------------------------------------------
V9_WARM_MM = 18


def build_nc_v9():
    fp8 = mybir.dt.float8e4
    nc = bacc.Bacc()
    bank8 = nc.declare_dram_parameter(
        "bank8", [128, V5_T, 2, 2, V5_CW], fp8, isOutput=False
    )
    tw8 = nc.declare_dram_parameter("tw8", [128, 2, 2, 2, 128], fp8,
                                    isOutput=False)
    outs_d = nc.declare_dram_parameter(
        "outs", [128, V5_T, 9], mybir.dt.float32, isOutput=True
    )

    with tile.TileContext(nc) as tc:
        with (
            tc.tile_pool(name="const", bufs=1) as constp,
            tc.tile_pool(name="bank", bufs=V6_PREFETCH + 1) as bankp,
            tc.tile_pool(name="cand", bufs=1) as candp,
            tc.tile_pool(name="ps", bufs=4, space="PSUM") as psp,
        ):
            tw = constp.tile([128, 2, 2, 2, 128], fp8)
            warm = constp.tile([128, 2, 256], fp8)
            nc.vector.memset(warm[:], 0.0)

            outs = candp.tile([128, V5_T, 9], mybir.dt.float32)

            bks = {}

            def issue_dma(su):
                bk = bankp.tile([128, 2, 2, V5_CW], fp8, tag="bank",
                                name="bk")
                if su == 0:
                    nc.sync.dma_start(bk[:, 0], bank8[:, 0, 0])
                    nc.scalar.dma_start(tw[:], tw8[:])
                    nc.scalar.dma_start(bk[:, 1], bank8[:, 0, 1])
                else:
                    eng = nc.sync if su % 2 == 0 else nc.gpsimd
                    eng.dma_start(bk[:], bank8[:, su])
                bks[su] = bk

            for su in range(V6_PREFETCH):
                issue_dma(su)

            # HAM warm-up: keep the PE busy through the DMA ramp so the
            # clock gate opens before the first real matmul
            wps = psp.tile([128, 2, 512], mybir.dt.float32, tag="ps",
                           name="wps")
            for _ in range(V9_WARM_MM):
                nc.tensor.matmul(
                    wps[:, 0, 0:256],
                    warm[:, :, 0:128],
                    warm[:],
                    start=True, stop=True,
                    perf_mode=mybir.MatmulPerfMode.DoubleRow,
                )

            for su in range(V5_T):
                if su + V6_PREFETCH < V5_T:
                    issue_dma(su + V6_PREFETCH)
                bk = bks.pop(su)
                pss = [psp.tile([128, 2, 512], mybir.dt.float32, tag="ps",
                                name="ps")
                       for h in range(2)]
                for h in range(2):
                    for j in range(2):
                        for c in range(2):
                            nc.tensor.matmul(
                                pss[h][:, c, 0:500],
                                tw[:, h, j],
                                bk[:, j, :, c * 500:(c + 1) * 500],
                                start=(j == 0),
                                stop=(j == 1),
                                perf_mode=mybir.MatmulPerfMode.DoubleRow,
                            )
                nc.scalar.activation(
                    pss[0][:, :, 0:500], pss[0][:, :, 0:500],
                    mybir.ActivationFunctionType.Exp,
                    scale=V4_ACT_SCALE,
                    accum_out=outs[:, su, 8:9],
                )
                nc.vector.max(outs[:, su, 0:8], pss[1][:, :, 0:500])

            nc.sync.dma_start(outs_d[:], outs[:])

    return nc


def _run_v9(exe, t, bank):
    global LAST_RESULTS
    bank8, tw8 = v6_pack_inputs(t, bank)
    concat = {"bank8": bank8, "tw8": tw8}
    results = exe([concat[n] for n in exe.in_names])
    LAST_RESULTS = results
    return _v7_select(results, t, bank)


# ---------------------------------------------------------------------------
# v8: v7 + pair-DMAs and h-major matmuls, from the v7 HW trace:
#   - Per-super 524KB DMAs (4000B per-partition runs) sustained only
#     ~169GB/s per ring; v5's 1MB pair transfers with 8000B runs hit
#     ~318GB/s.  v8 reloads the pair layout: one DMA per 2 super-tiles,
#     alternating sync/GpSimd rings (first pair split su0->sync,
#     su1->ACT-ring so the PE starts early), consumed per super.
#   - v7 issued matmuls j-major, so a super's h0 PSUM tile was only
#     complete ~1.3us into its 1.71us MM phase and the 4-tile PSUM pool
#     left PE ~0.3us/super of recycle slack.  v8 issues h-major (h0's two
#     j-planes first): h0's consumer starts at 0.86us, roughly 1.2us of
#     slack per tile.
# ---------------------------------------------------------------------------


def build_nc_v8():
    fp8 = mybir.dt.float8e4
    nc = bacc.Bacc()
    # [p, g, s, j, i, n] = b8[(g*2+s)*1000 + n, j*256 + i*128 + p]
    bank8 = nc.declare_dram_parameter(
        "bank8", [128, V5_NG, V5_G, 2, 2, V5_CW], fp8, isOutput=False
    )
    tw8 = nc.declare_dram_parameter("tw8", [128, 2, 2, 2, 128], fp8,
                                    isOutput=False)
    outs_d = nc.declare_dram_parameter(
        "outs", [128, V5_T, 9], mybir.dt.float32, isOutput=True
    )

    with tile.TileContext(nc) as tc:
        with (
            tc.tile_pool(name="const", bufs=1) as constp,
            tc.tile_pool(name="bank", bufs=3) as bankp,
            tc.tile_pool(name="cand", bufs=1) as candp,
            tc.tile_pool(name="ps", bufs=4, space="PSUM") as psp,
        ):
            tw = constp.tile([128, 2, 2, 2, 128], fp8)

            outs = candp.tile([128, V5_T, 9], mybir.dt.float32)

            bkp = {}

            def issue_pair(g):
                bk = bankp.tile([128, V5_G, 2, 2, V5_CW], fp8, tag="bank",
                                name="bk")
                if g == 0:
                    # cold start: su0/su1 split across the sync and GpSimd
                    # rings, tw (131KB) alone on the ACT ring
                    nc.sync.dma_start(bk[:, 0], bank8[:, 0, 0])
                    nc.scalar.dma_start(tw[:], tw8[:])
                    nc.gpsimd.dma_start(bk[:, 1], bank8[:, 0, 1])
                else:
                    eng = nc.sync if g % 2 == 1 else nc.gpsimd
                    eng.dma_start(bk[:], bank8[:, g])
                bkp[g] = bk

            issue_pair(0)
            issue_pair(1)

            for su in range(V5_T):
                g, s = divmod(su, V5_G)
                if s == 0 and g + 2 < V5_NG:
                    issue_pair(g + 2)
                bk = bkp[g]
                pss = [psp.tile([128, 2, 512], mybir.dt.float32, tag="ps",
                                name="ps")
                       for h in range(2)]
                for h in range(2):
                    for j in range(2):
                        for c in range(2):
                            nc.tensor.matmul(
                                pss[h][:, c, 0:500],
                                tw[:, h, j],
                                bk[:, s, j, :, c * 500:(c + 1) * 500],
                                start=(j == 0),
                                stop=(j == 1),
                                perf_mode=mybir.MatmulPerfMode.DoubleRow,
                            )
                nc.scalar.activation(
                    pss[0][:, :, 0:500], pss[0][:, :, 0:500],
                    mybir.ActivationFunctionType.Exp,
                    scale=V4_ACT_SCALE,
                    accum_out=outs[:, su, 8:9],
                )
                nc.vector.max(outs[:, su, 0:8], pss[1][:, :, 0:500])

            nc.sync.dma_start(outs_d[:], outs[:])

    return nc


def _run_v8(exe, t, bank):
    global LAST_RESULTS
    bank8, tw8 = v5_pack_inputs(t, bank)
    concat = {"bank8": bank8, "tw8": tw8}
    results = exe([concat[n] for n in exe.in_names])
    LAST_RESULTS = results
    return _v7_select(results, t, bank)


def _v7_select(results, t, bank):
    """Shared v7/v8 host epilogue (packed [128, 16, 9] outs)."""
    proxy = np.empty((B, N_CORES, V5_T), np.float32)
    for m, r in enumerate(results):
        o = r["outs"]                                     # [128, 16, 9]
        proxy[:128, m, :] = np.log(
            np.maximum(o[:, :, 8], 1e-30)) / V4_SEFF
        proxy[128:, m, :] = o[:, :, 0] / (V4_S_BANK * V4_S_T)
    proxy = proxy.reshape(B, N_CORES * V5_T)

    sel = np.argpartition(-proxy, V5_W - 1, axis=1)[:, :V5_W]
    t_n = t / np.maximum(np.linalg.norm(t, axis=1, keepdims=True), EPS)
    span = np.arange(V5_CW, dtype=np.int64)
    top5 = np.empty((B, TOPK), np.int64)
    for b in range(B):
        widx = (sel[b].astype(np.int64)[:, None] * V5_CW
                + span[None, :]).ravel()
        wsims = bank[widx] @ t_n[b]
        o = np.lexsort((widx, -wsims))
        top5[b] = widx[o[:TOPK]]
    return top5


# ---------------------------------------------------------------------------
# v7: v6 + trigger/tail/interleave fixes, from the v6 HW trace:
#   - Each HWDGE dma_start costs ~600ns on its issuing engine: the 8-way
#     split first load burned ~4us of ramp, and 4 output DMAs ~2.4us of
#     tail.  v7 issues one DMA per super-tile (even: sync ring, odd: the
#     otherwise-idle GpSimd SWDGE ring, keeping the ACT queue clean) and
#     packs all outputs into one [128, 16, 9] tile (single DMA).
#   - v6 ran both of a super's consumers serially on one engine (2.7us)
#     while the other idled.  v7 sends half h=0 to ScalarE-LSE and h=1 to
#     DVE-max8 for EVERY super: the two run in parallel (~1.4us/super).
#     Each B-row is scored by a single proxy type for all its chunks
#     (rows 0-127: LSE, rows 128-255: max8) -- both validated offline.
# ---------------------------------------------------------------------------
V7_PREFETCH = 6


def build_nc_v7():
    fp8 = mybir.dt.float8e4
    nc = bacc.Bacc()
    bank8 = nc.declare_dram_parameter(
        "bank8", [128, V5_T, 2, 2, V5_CW], fp8, isOutput=False
    )
    tw8 = nc.declare_dram_parameter("tw8", [128, 2, 2, 2, 128], fp8,
                                    isOutput=False)
    # [p, su, 0:8] = top-8 of super su for row 128+p (DVE half);
    # [p, su, 8]   = sum(exp(0.5*psum)) of super su for row p (ACT half)
    outs_d = nc.declare_dram_parameter(
        "outs", [128, V5_T, 9], mybir.dt.float32, isOutput=True
    )

    with tile.TileContext(nc) as tc:
        with (
            tc.tile_pool(name="const", bufs=1) as constp,
            tc.tile_pool(name="bank", bufs=V7_PREFETCH + 1) as bankp,
            tc.tile_pool(name="cand", bufs=1) as candp,
            tc.tile_pool(name="ps", bufs=4, space="PSUM") as psp,
        ):
            tw = constp.tile([128, 2, 2, 2, 128], fp8)
            warm = constp.tile([128, 2, 256], fp8)
            nc.vector.memset(warm[:], 0.0)

            outs = candp.tile([128, V5_T, 9], mybir.dt.float32)

            bks = {}

            def issue_dma(su):
                bk = bankp.tile([128, 2, 2, V5_CW], fp8, tag="bank",
                                name="bk")
                if su <= 1:
                    # cold-start: halve the first two supers across the two
                    # HWDGE rings; tw (131KB) rides the otherwise-idle ACT
                    # ring.  Both su1 halves land ~1.5us before a whole-su1
                    # sync transfer would.
                    if su == 0:
                        nc.sync.dma_start(bk[:, 0], bank8[:, 0, 0])
                        nc.scalar.dma_start(tw[:], tw8[:])
                        nc.scalar.dma_start(bk[:, 1], bank8[:, 0, 1])
                    else:
                        nc.sync.dma_start(bk[:, 0], bank8[:, 1, 0])
                        nc.scalar.dma_start(bk[:, 1], bank8[:, 1, 1])
                else:
                    # 11/5 ring split: the SWDGE (GpSimd) ring sustains
                    # only ~115GB/s vs sync HWDGE's ~180-230 warm, and
                    # carrying half the bytes it pinned the last super's
                    # arrival (and thus the last matmul) at ~51.5k
                    # su1 must beat GpSimd's slow (~6us) first SWDGE
                    # transfer: sync's 2nd lands ~2us earlier
                    eng = (nc.gpsimd if su >= 2 and (su - 2) % 3 == 0
                           else nc.sync)
                    eng.dma_start(bk[:], bank8[:, su])
                bks[su] = bk

            for su in range(V7_PREFETCH):
                issue_dma(su)

            # start-only HAM warm-up: dummy DoubleRow MMs on the zeroed
            # tile keep the clock gate open through the ~3us DMA ramp so
            # su0's real matmuls issue at the full 213ns rate.  (A pad
            # block BETWEEN supers stalls on scheduler semaphores -- v16.)
            wps = psp.tile([128, 2, 512], mybir.dt.float32, tag="ps",
                           name="wps")
            for _ in range(16):
                nc.tensor.matmul(
                    wps[:, 0, 0:256],
                    warm[:, :, 0:128],
                    warm[:],
                    start=True, stop=True,
                    perf_mode=mybir.MatmulPerfMode.DoubleRow,
                )

            for su in range(V5_T):
                if su + V7_PREFETCH < V5_T:
                    issue_dma(su + V7_PREFETCH)
                bk = bks.pop(su)
                pss = [psp.tile([128, 2, 512], mybir.dt.float32, tag="ps",
                                name="ps")
                       for h in range(2)]
                for j in range(2):
                    for h in range(2):
                        for c in range(2):
                            nc.tensor.matmul(
                                pss[h][:, c, 0:500],
                                tw[:, h, j],
                                bk[:, j, :, c * 500:(c + 1) * 500],
                                start=(j == 0),
                                stop=(j == 1),
                                perf_mode=mybir.MatmulPerfMode.DoubleRow,
                            )
                nc.scalar.activation(
                    pss[0][:, :, 0:500], pss[0][:, :, 0:500],
                    mybir.ActivationFunctionType.Exp,
                    scale=V4_ACT_SCALE,
                    accum_out=outs[:, su, 8:9],
                )
                nc.vector.max(outs[:, su, 0:8], pss[1][:, :, 0:500])

            nc.sync.dma_start(outs_d[:], outs[:])

    return nc


def _run_v7(exe, t, bank):
    global LAST_RESULTS
    bank8, tw8 = v6_pack_inputs(t, bank)
    concat = {"bank8": bank8, "tw8": tw8}
    results = exe([concat[n] for n in exe.in_names])
    LAST_RESULTS = results

    # proxy per (row, core, super): rows 0-127 from the LSE scores,
    # rows 128-255 from the max8 values -- both ~ chunk max in sim units
    proxy = np.empty((B, N_CORES, V5_T), np.float32)
    for m, r in enumerate(results):
        o = r["outs"]                                     # [128, 16, 9]
        proxy[:128, m, :] = np.log(
            np.maximum(o[:, :, 8], 1e-30)) / V4_SEFF
        proxy[128:, m, :] = o[:, :, 0] / (V4_S_BANK * V4_S_T)
    proxy = proxy.reshape(B, N_CORES * V5_T)

    sel = np.argpartition(-proxy, V5_W - 1, axis=1)[:, :V5_W]
    t_n = t / np.maximum(np.linalg.norm(t, axis=1, keepdims=True), EPS)
    span = np.arange(V5_CW, dtype=np.int64)
    top5 = np.empty((B, TOPK), np.int64)
    for b in range(B):
        widx = (sel[b].astype(np.int64)[:, None] * V5_CW
                + span[None, :]).ravel()
        wsims = bank[widx] @ t_n[b]
        o = np.lexsort((widx, -wsims))
        top5[b] = widx[o[:TOPK]]
    return top5


# ---------------------------------------------------------------------------
# v6: v5 + deep software pipelining, from the v5 HW trace:
#   - Warm PE issues a DoubleRow MM every 210ns (fp8 peak, 157 TF/s; 27us
#     for all 128), but ~half the kernel ran HAM-throttled (420ns/MM)
#     because of 4-5us PE idle gaps at group boundaries: the 4-psum-tile
#     groups consumed ALL of PSUM (no cross-group overlap), and odd-group
#     DMA triggers sat behind the previous group's ACT instructions in the
#     ACT queue.
#   - v6: one super-tile per step (2 psum tiles -> two steps in flight),
#     per-super 524KB DMAs issued 3 steps ahead, alternating the sync/ACT
#     HWDGE rings, with the trigger emitted BEFORE the step's consumers.
# ---------------------------------------------------------------------------
V6_PREFETCH = 3


def build_nc_v6():
    fp8 = mybir.dt.float8e4
    act_set = set(v5_act_supers())
    n_act = len(act_set)
    n_dve = V5_T - n_act
    nc = bacc.Bacc()
    # [p, s, j, i, n] = b8[s*1000 + n, j*256 + i*128 + p]
    bank8 = nc.declare_dram_parameter(
        "bank8", [128, V5_T, 2, 2, V5_CW], fp8, isOutput=False
    )
    tw8 = nc.declare_dram_parameter("tw8", [128, 2, 2, 2, 128], fp8,
                                    isOutput=False)
    cand_v = nc.declare_dram_parameter(
        "cand_v", [B, n_dve, 8], mybir.dt.float32, isOutput=True
    )
    scores = nc.declare_dram_parameter(
        "scores", [B, n_act], mybir.dt.float32, isOutput=True
    )

    with tile.TileContext(nc) as tc:
        with (
            tc.tile_pool(name="const", bufs=1) as constp,
            tc.tile_pool(name="bank", bufs=V6_PREFETCH + 1) as bankp,
            tc.tile_pool(name="cand", bufs=1) as candp,
            tc.tile_pool(name="ps", bufs=4, space="PSUM") as psp,
        ):
            tw = constp.tile([128, 2, 2, 2, 128], fp8)
            nc.sync.dma_start(tw[:], tw8[:])

            cands = [
                candp.tile([128, n_dve, 8], mybir.dt.float32,
                           tag=f"c{h}", name=f"cands{h}")
                for h in range(2)
            ]
            scs = [
                candp.tile([128, n_act], mybir.dt.float32,
                           tag=f"s{h}", name=f"scores{h}")
                for h in range(2)
            ]

            bks = {}

            def issue_dma(su):
                bk = bankp.tile([128, 2, 2, V5_CW], fp8, tag="bank",
                                name="bk")
                if su == 0:
                    # fine-grained first load: first matmul starts after 1/4
                    for j in range(2):
                        for i in range(2):
                            nc.sync.dma_start(bk[:, j, i], bank8[:, 0, j, i])
                else:
                    eng = nc.sync if su % 2 == 0 else nc.scalar
                    eng.dma_start(bk[:], bank8[:, su])
                bks[su] = bk

            for su in range(V6_PREFETCH):
                issue_dma(su)

            d_idx = a_idx = 0
            for su in range(V5_T):
                if su + V6_PREFETCH < V5_T:
                    issue_dma(su + V6_PREFETCH)
                bk = bks.pop(su)
                pss = [psp.tile([128, 2, 512], mybir.dt.float32, tag="ps",
                                name="ps")
                       for h in range(2)]
                for j in range(2):
                    for h in range(2):
                        for c in range(2):
                            nc.tensor.matmul(
                                pss[h][:, c, 0:500],
                                tw[:, h, j],
                                bk[:, j, :, c * 500:(c + 1) * 500],
                                start=(j == 0),
                                stop=(j == 1),
                                perf_mode=mybir.MatmulPerfMode.DoubleRow,
                            )
                for h in range(2):
                    view = pss[h][:, :, 0:500]
                    if su in act_set:
                        nc.scalar.activation(
                            view, view,
                            mybir.ActivationFunctionType.Exp,
                            scale=V4_ACT_SCALE,
                            accum_out=scs[h][:, a_idx:a_idx + 1],
                        )
                    else:
                        nc.vector.max(cands[h][:, d_idx, :], view)
                if su in act_set:
                    a_idx += 1
                else:
                    d_idx += 1

            for h in range(2):
                nc.sync.dma_start(cand_v[h * 128:(h + 1) * 128, :], cands[h][:])
                nc.sync.dma_start(scores[h * 128:(h + 1) * 128, :], scs[h][:])

    return nc


def v6_pack_inputs(t, bank):
    """Host-side fp8 packing for v6 (per-super-tile layout)."""
    f8 = ml_dtypes.float8_e4m3
    t_n = t / np.maximum(np.linalg.norm(t, axis=1, keepdims=True), EPS)
    t8 = (t_n * V4_S_T).astype(f8)
    b8 = (bank * V4_S_BANK).astype(f8)
    bank8 = (
        b8.reshape(N_CORES, V5_T, V5_CW, 2, 2, 128)     # m,s,n,j,i,p
        .transpose(0, 5, 1, 3, 4, 2)                    # m,p,s,j,i,n
        .reshape(N_CORES * 128, V5_T, 2, 2, V5_CW)
    )
    tw8_1 = (
        t8.reshape(2, 128, 2, 2, 128)                   # h,m,j,i,p
        .transpose(4, 0, 2, 3, 1)                       # p,h,j,i,m
    )
    tw8 = np.concatenate([tw8_1] * N_CORES, axis=0)
    return np.ascontiguousarray(bank8), np.ascontiguousarray(tw8)


def _run_v6(exe, t, bank):
    global LAST_RESULTS
    bank8, tw8 = v6_pack_inputs(t, bank)
    concat = {"bank8": bank8, "tw8": tw8}
    results = exe([concat[n] for n in exe.in_names])
    LAST_RESULTS = results
    return _v5_select(results, t, bank)


def _v5_select(results, t, bank):
    """Shared v5/v6 host epilogue: proxies -> windows -> exact top-5."""
    act_supers = v5_act_supers()
    dve_supers = [s for s in range(V5_T) if s not in set(act_supers)]
    proxy = np.empty((B, N_CORES, V5_T), np.float32)
    for m, r in enumerate(results):
        proxy[:, m, dve_supers] = r["cand_v"][:, :, 0] / (V4_S_BANK * V4_S_T)
        proxy[:, m, act_supers] = np.log(
            np.maximum(r["scores"], 1e-30)) / V4_SEFF
    proxy = proxy.reshape(B, N_CORES * V5_T)

    sel = np.argpartition(-proxy, V5_W - 1, axis=1)[:, :V5_W]
    t_n = t / np.maximum(np.linalg.norm(t, axis=1, keepdims=True), EPS)
    span = np.arange(V5_CW, dtype=np.int64)
    top5 = np.empty((B, TOPK), np.int64)
    for b in range(B):
        widx = (sel[b].astype(np.int64)[:, None] * V5_CW
                + span[None, :]).ravel()
        wsims = bank[widx] @ t_n[b]
        o = np.lexsort((widx, -wsims))
        top5[b] = widx[o[:TOPK]]
    return top5


# "v1": two DVE scans per chunk (max8 + max_index) -- simplest, and the
#       faster schedule under the TRN2 instruction cost model (87.8us vs
#       109.6us predicted per core; DVE-bound).
# "v2": tagged single-scan -- one DVE max8 pass; the PE quantizes sims
#       in-PSUM (+4/-4 rank-1s) and adds a sub-quantum subchunk tag that
#       the host decodes, trading DVE time for PE time. Better if real
#       silicon streams bf16 matmuls near the documented 131ns/MM rate.
# "v3": v1's matmul+max8 pipeline with NO max_index pass at all -- the
#       candidate slot already identifies the 500-wide chunk, so the host
#       recomputes the <=8 best chunks per row (~1 GFLOP) to recover exact
#       indices. Halves DVE work; model-predicted 70.5us vs 84.5us (v1).
# "v4": fp8 DoubleRow matmuls + ACT/DVE split chunk scoring from PSUM --
#       see the block comment above build_nc_v4.
# "v5": v4 + 1MB dual-ring DMAs, stationary-swept matmul groups, 1000-wide
#       scoring units -- see the block comment above build_nc_v5.
# v1-v3 validated on the fixed inputs (HW): v1 loss rel err 4.9e-5,
# v2 5.3e-6, v3 4.9e-5; purity exact in all.  v4 rel err 0.0 (HW).
MODE = "v7"

_NC_CACHE = {}


def _get_nc():
    key = (MODE, DTYPE)
    if key not in _NC_CACHE:
        if MODE == "v10":
            nc = build_nc_v10()
        elif MODE == "v9":
            nc = build_nc_v9()
        elif MODE == "v8":
            nc = build_nc_v8()
        elif MODE == "v7":
            nc = build_nc_v7()
        elif MODE == "v6":
            nc = build_nc_v6()
        elif MODE == "v5":
            nc = build_nc_v5()
        elif MODE == "v4":
            nc = build_nc_v4()
        elif MODE == "v2":
            nc = build_nc_v2()
        elif MODE == "v3":
            nc = build_nc(DTYPE, with_index=False)
        else:
            nc = build_nc(DTYPE)
        nc.finalize()
        _NC_CACHE[key] = nc
    return _NC_CACHE[key]


class _SpmdExec:
    """Cached jitted shard_map over the bass_exec custom call.

    Mirrors bass2jax.run_bass_via_pjrt's multi-core path but builds the
    jitted executable once, so repeated calls skip retrace/recompile.
    """

    def __init__(self, nc):
        bass2jax.install_neuronx_cc_hook()
        part_name = (
            nc.partition_id_tensor.name if nc.partition_id_tensor else None
        )
        in_names, out_names, out_avals = [], [], []
        for alloc in nc.m.functions[0].allocations:
            if not isinstance(alloc, mybir.MemoryLocationSet):
                continue
            name = alloc.memorylocations[0].name
            if alloc.kind == "ExternalInput":
                if name != part_name:
                    in_names.append(name)
            elif alloc.kind == "ExternalOutput":
                out_names.append(name)
                out_avals.append(
                    jax.core.ShapedArray(
                        tuple(alloc.tensor_shape), mybir.dt.np(alloc.dtype)
                    )
                )
        self.in_names = list(in_names)
        self.out_names = out_names
        self.out_avals = out_avals
        n_params = len(in_names)
        n_outs = len(out_names)
        bind_names = in_names + out_names
        if part_name is not None:
            bind_names = bind_names + [part_name]
        bind_names = tuple(bind_names)

        def _body(*args):
            operands = list(args)
            if part_name is not None:
                operands.append(bass2jax.partition_id_tensor())
            outs = bass2jax._bass_exec_p.bind(
                *operands,
                out_avals=tuple(out_avals),
                in_names=bind_names,
                out_names=tuple(out_names),
                lowering_input_output_aliases=(),
                sim_require_finite=True,
                sim_require_nnan=True,
                nc=nc,
            )
            return tuple(outs)

        devices = jax.devices()[:N_CORES]
        self.mesh = Mesh(np.asarray(devices), ("core",))
        in_specs = (PartitionSpec("core"),) * (n_params + n_outs)
        out_specs = (PartitionSpec("core"),) * n_outs
        self.fn = jax.jit(
            shard_map(
                _body,
                mesh=self.mesh,
                in_specs=in_specs,
                out_specs=out_specs,
                check_rep=False,
            ),
            donate_argnums=tuple(range(n_params, n_params + n_outs)),
            keep_unused=True,
        )

    def zero_outs(self):
        return [
            np.zeros((N_CORES * a.shape[0], *a.shape[1:]), a.dtype)
            for a in self.out_avals
        ]

    def __call__(self, concat_inputs):
        """concat_inputs: list matching in_names, each (N_CORES*dim0, ...)."""
        out_arrs = self.fn(*concat_inputs, *self.zero_outs())
        return [
            {
                name: np.asarray(out_arrs[i]).reshape(
                    N_CORES, *self.out_avals[i].shape
                )[c]
                for i, name in enumerate(self.out_names)
            }
            for c in range(N_CORES)
        ]


_EXEC_CACHE = {}


def _get_exec():
    key = (MODE, DTYPE)
    if key not in _EXEC_CACHE:
        _EXEC_CACHE[key] = _SpmdExec(_get_nc())
    return _EXEC_CACHE[key]


def _np_dtype(dtype):
    return ml_dtypes.bfloat16 if dtype == mybir.dt.bfloat16 else np.float32


def _run_v1(exe, bank_sh, t, tT):
    """max8 + max_index path: returns per-row global top-5 indices."""
    global LAST_RESULTS
    np_dt = _np_dtype(DTYPE)
    tT_c = tT.astype(np_dt)
    concat = {
        "bankT": bank_sh,
        "tT": np.concatenate([tT_c] * N_CORES, axis=0),
    }
    results = exe([concat[n] for n in exe.in_names])
    LAST_RESULTS = results

    vals = np.stack([r["cand_v"] for r in results], axis=1)
    idx_l = np.stack(
        [r["cand_i"].astype(np.int64) for r in results], axis=1
    )
    groups = groups_for(KL)
    gbase = np.concatenate([[0], np.cumsum(groups)[:-1]]).astype(np.int64)
    base = (
        np.arange(N_CORES, dtype=np.int64)[None, :, None] * KL
        + np.repeat(gbase, 8)[None, None, :]
    )
    gidx = (idx_l + base).reshape(B, -1)            # global indices
    vals = vals.reshape(B, -1)                      # raw sim_t

    # Emulate the reference's comparison domain: fp32 dist_t with per-row
    # 1/||t_b|| folded back in; ties break toward the lowest global index.
    inv_t = 1.0 / np.maximum(np.linalg.norm(t, axis=1), EPS)   # [B]
    dist32 = (2.0 - 2.0 * vals * inv_t[:, None]).astype(np.float32)
    top5 = np.empty((B, TOPK), np.int64)
    for b in range(B):
        order = np.lexsort((gidx[b], dist32[b]))
        top5[b] = gidx[b][order[:TOPK]]
    return top5


N_WINDOWS = 10  # per-row candidate windows recomputed exactly on the host


def _run_v2(exe, bank_sh, t, bank):
    """Tagged single-scan path: returns per-row global top-5 indices."""
    global LAST_RESULTS
    bf = ml_dtypes.bfloat16
    t_n = t / np.maximum(np.linalg.norm(t, axis=1, keepdims=True), EPS)
    tw = np.ascontiguousarray((t_n * SIM_SCALE).T).astype(bf)   # [C, B]
    consts = _make_consts()
    concat = {
        "bankT": bank_sh,
        "tT": np.concatenate([tw] * N_CORES, axis=0),
        "consts": np.concatenate([consts] * N_CORES, axis=0),
    }
    results = exe([concat[n] for n in exe.in_names])
    LAST_RESULTS = results

    # packed candidates [B, N_CORES, NCAND]
    packed = np.stack([r["cand_v"] for r in results], axis=1)
    pk = packed.reshape(B, -1).astype(np.float64)    # [B, 512]
    # packed = q(sim) + id*2^-25 with q a multiple of 2^-21 (positive sims)
    y = np.round(pk / TAG_EPS).astype(np.int64)      # exact integer
    dec_id = np.mod(y, N_SUB)
    qsim = pk - dec_id * TAG_EPS                     # quantized scaled sim
    # window start (global bank row) per candidate
    cores = np.repeat(np.arange(N_CORES, dtype=np.int64), NCAND)[None, :]
    groups = np.tile(
        np.repeat(np.arange(N_GRP, dtype=np.int64), 8), N_CORES
    )[None, :]
    wstart = cores * KL + groups * CHUNK + dec_id * SUB   # [B, 512]

    # top-N_WINDOWS candidates per row by qsim; recompute those 125-wide
    # windows exactly (fp32 over the bf16-cast operands, matching the
    # device's computation up to summation order) and take the exact top-5.
    order = np.argsort(-qsim, axis=1, kind="stable")[:, :N_WINDOWS]
    sel_start = np.take_along_axis(wstart, order, axis=1)     # [B, W]

    bank_bf = bank.astype(bf).astype(np.float32)              # [K, C]
    t_bf = (t_n * SIM_SCALE).astype(bf).astype(np.float32)    # [B, C]
    flat_idx = (sel_start[:, :, None] +
                np.arange(SUB, dtype=np.int64)[None, None, :])  # [B, W, SUB]
    rows = bank_bf[flat_idx.reshape(-1)].reshape(B, N_WINDOWS * SUB, C)
    wsims = np.einsum("bkc,bc->bk", rows, t_bf)               # [B, W*SUB]
    widx = flat_idx.reshape(B, -1)                            # [B, W*SUB]

    top5 = np.empty((B, TOPK), np.int64)
    for b in range(B):
        # windows may overlap -> dedupe indices, keep exact values
        o = np.lexsort((widx[b], -wsims[b]))
        seen, picks = set(), []
        for i in o:
            gi = widx[b, i]
            if gi in seen:
                continue
            seen.add(gi)
            picks.append(gi)
            if len(picks) == TOPK:
                break
        top5[b] = picks
    return top5


def _run_v3(exe, bank_sh, t, bank):
    """Index-free path: per-chunk top-8 values only (exact fp32, a
    deterministic superset of the per-chunk top-5); the host recovers
    indices by recomputing the <=8 best 500-wide chunks per row."""
    global LAST_RESULTS
    np_dt = _np_dtype(DTYPE)
    tT_c = np.ascontiguousarray(t.T).astype(np_dt)
    concat = {
        "bankT": bank_sh,
        "tT": np.concatenate([tT_c] * N_CORES, axis=0),
    }
    results = exe([concat[n] for n in exe.in_names])
    LAST_RESULTS = results

    n_grp = KL // KT                                 # 32 chunks of 500
    vals = np.stack([r["cand_v"] for r in results], axis=1)
    vals = vals.reshape(B, -1)                       # [B, 8*32*8=2048]
    # candidate slot -> global chunk start (chunk known from position)
    cores = np.repeat(np.arange(N_CORES, dtype=np.int64), 8 * n_grp)
    chunks = np.tile(np.repeat(np.arange(n_grp, dtype=np.int64), 8), N_CORES)
    wstart = (cores * KL + chunks * KT)[None, :]     # [1, 2048]

    # every true top-5 element is a candidate with a top-5 value, so the
    # top-8 candidate windows per row cover them deterministically
    order = np.argsort(-vals, axis=1, kind="stable")[:, :8]
    sel = np.take_along_axis(np.broadcast_to(wstart, vals.shape),
                             order, axis=1)          # [B, 8]

    bf = ml_dtypes.bfloat16
    bank_bf = bank.astype(bf).astype(np.float32)     # [K, C]
    t_bf = t.astype(bf).astype(np.float32)           # [B, C]
    top5 = np.empty((B, TOPK), np.int64)
    span = np.arange(KT, dtype=np.int64)
    for b in range(B):
        starts = np.unique(sel[b])
        widx = (starts[:, None] + span[None, :]).reshape(-1)
        wsims = bank_bf[widx] @ t_bf[b]              # exact bf16-input sims
        o = np.lexsort((widx, -wsims))
        top5[b] = widx[o[:TOPK]]
    return top5


def kernel(query, current_target, queue, labels, labels_queue):
    query = np.asarray(query, np.float32)
    t = np.asarray(current_target, np.float32)
    queue_f = np.asarray(queue, np.float32)
    labels = np.asarray(labels)
    labels_queue = np.asarray(labels_queue)

    # Host prep: normalize bank rows (fp32, matching reference), transpose.
    norms = np.maximum(np.linalg.norm(queue_f, axis=1), EPS)
    bank = queue_f / norms[:, None]                 # [K, C], normalized
    tT = np.ascontiguousarray(t.T)                  # [C, B]

    np_dt = _np_dtype(DTYPE)
    exe = _get_exec()
    if MODE == "v10":
        top5 = _run_v10(exe, t, bank)
    elif MODE == "v9":
        top5 = _run_v9(exe, t, bank)
    elif MODE == "v8":
        top5 = _run_v8(exe, t, bank)
    elif MODE == "v7":
        top5 = _run_v7(exe, t, bank)
    elif MODE == "v6":
        top5 = _run_v6(exe, t, bank)
    elif MODE == "v5":
        top5 = _run_v5(exe, t, bank)
    elif MODE == "v4":
        top5 = _run_v4(exe, t, bank)
    else:
        # [8*C, KL]: core m's shard (rows m*C..(m+1)*C) is
        # bank[m*KL:(m+1)*KL].T
        bank_sh = np.ascontiguousarray(
            bank.reshape(N_CORES, KL, C).transpose(0, 2, 1)
        ).astype(np_dt).reshape(N_CORES * C, KL)
        if MODE == "v2":
            top5 = _run_v2(exe, bank_sh, t, bank)
        elif MODE == "v3":
            top5 = _run_v3(exe, bank_sh, t, bank)
        else:
            top5 = _run_v1(exe, bank_sh, t, tT)

    # dist_q at the selected indices + purity.
    q_norm = query / np.maximum(
        np.linalg.norm(query, axis=1, keepdims=True), EPS
    )
    rows = bank[top5.reshape(-1)].reshape(B, TOPK, C)          # normalized
    nn_dist_q = 2.0 - 2.0 * np.einsum(
        "bjc,bc->bj", rows.astype(np.float64), q_norm.astype(np.float64)
    )
    loss = nn_dist_q.mean()
    matches = labels_queue[top5] == labels[:, None]
    purity = matches.mean()
    return (np.float32(loss), np.float32(purity))



# revision 38
# speedup vs baseline: 1.0414x; 1.0414x over previous
"""Trainium2 Bass kernel for nn_MeanShift (retrieval_knn).

Full-input contract: kernel(**inputs) -> (loss, purity).

Shipped design (MODE="v7"; earlier modes kept for reference):
  - Bank (K=128000) sharded across 8 cores (16000 rows each); targets
    replicated.  Host casts inputs to fp8e4 with power-of-2 scales
    (bank x8, normalized t x64) and packs a per-partition-contiguous
    layout per 1000-row super-tile.
  - Per core, 16 super-tiles: per-super 524KB DMAs (even: sync HWDGE
    ring, odd: GpSimd SWDGE ring, triggers prefetched 3 ahead), fp8
    DoubleRow matmuls (2x128 contraction planes, 0.5 cyc/row -- measured
    at the 157 TF/s fp8 peak, one 500-col MM issued per 213ns), PSUM
    [128, 2, 512] tiles.
  - Chunk scoring reads PSUM directly, split across two engines per
    super: half h=0 -> ScalarE Exp activation in-place with accum_out
    (log-sum-exp score, s_eff=256), half h=1 -> DVE max8.  One packed
    [128, 16, 9] output tile, single DMA.
  - Host epilogue: per-row proxies over all 128 chunk scores (ln(score)/
    256 or cand0/512 ~ chunk max in cosine units), top-16 1000-wide
    windows recomputed exactly in fp32, global top-5 with reference
    tie-breaking (lowest index), then dist_q/loss/purity.

Selection correctness: on the fixed inputs the worst true-top-5 chunk
ranks 10th of 128 by either proxy (fp8 sim noise sigma=1.7e-3), so W=16
windows cover every row with margin; the exact-fp32 window recompute
then reproduces the reference answer bit-for-bit (rel err 0.0 on HW).

DMA ring split (v14-v17 tuning): the GpSimd SWDGE ring sustains only
~115GB/s vs the sync HWDGE ring's ~180-230GB/s warm, so supers split
11 (sync) / 5 (gpsimd, every 3rd from su2).  su0 AND su1 are halved
across the sync + ACT HWDGE rings (the ACT ring is free until ScalarE's
first consumer), and 14 dummy DoubleRow matmuls on a zeroed tile bridge
the ~3us DMA ramp so the HAM clock gate is open when su0's real
matmuls issue (start-only: pad blocks BETWEEN supers stall on
scheduler semaphores).

Measured per-core NEFF exec (neuron-profile): 47.5-50.3us; engine
floors: PE 27.3us (fp8 peak), DMA ~24us, ACT ~23us, DVE ~19us, plus
~3us ramp and ~10us fixed BIR exit barrier.
"""

import numpy as np
import ml_dtypes

import jax
from jax.experimental.shard_map import shard_map
from jax.sharding import Mesh, PartitionSpec

import concourse.bass as bass
import concourse.bacc as bacc
import concourse.mybir as mybir
import concourse.tile as tile
from concourse import bass2jax

N_CORES = 8
B = 256          # batch (rows of query/current_target)
C = 512          # feature dim
K = 128000       # memory bank size
KL = K // N_CORES  # 16000 bank rows per core
KT = 500         # matmul k-tile width (PSUM bank holds 512 fp32)
GRP = 4          # k-tiles per max-scan chunk (v2 path)
CHUNK = KT * GRP   # 2000 elements per DVE max8 scan (v2 path)
N_GRP = KL // CHUNK  # 8 scan chunks per core (v2 path)
NCAND = 8 * N_GRP    # 64 candidates per row per core (v2 path)
TOPK = 5
EPS = 1e-12


def groups_for(kl):
    """v1 scan-chunk widths. Six 500-wide leading groups cut the DVE
    start-up ramp; 1000-wide steady-state chunks schedule tighter than
    2000 (TimelineSim: 84.5us vs 87.8us per core for kl=16000)."""
    if kl >= 4000 and (kl - 3000) % 1000 == 0:
        return [500] * 6 + [1000] * ((kl - 3000) // 1000)
    assert kl % KT == 0
    return [KT] * (kl // KT)

# bfloat16 halves DMA + PE time; fp32 is the accuracy-safe fallback.
# Validated on the fixed inputs: bf16 changes 15/256 rows' top-5 with min
# 5th/6th sim gap 2.9e-4 (>> HW accumulation noise), loss rel err 4.8e-5,
# purity identical (0.0) -- well inside the 2e-2 gate.
DTYPE = mybir.dt.bfloat16

# v2 (tagged single-scan) constants. Device computes sims scaled to
# |sim| <= 0.25 (host passes t_norm/4; actual |sim| ~ 0.05). Per 500-wide
# matmul tile the PE appends three rank-1 accumulations, in order:
#   +4.0   -- rounds sim onto the 2^-21 grid (exponent pinned at 2^2)
#   -4.0   -- Sterbenz-exact unshift, psum = q(sim), a 2^-21 multiple
#   +id*2^-25, id in [0,16) the 125-wide subchunk of the column -- exact
#          (ulp <= 2^-26 for |q| < 0.25), and SUB-quantum, so packed
#          ordering matches q(sim) ordering to within one quantum.
# One max8 scan returns packed = q(sim) + id*2^-25; the host decodes
# id = (packed/2^-25) mod 16 (q/2^-25 is a multiple of 16 for the
# positive sims that matter) and re-derives exact values by recomputing
# the winners' 125-wide windows.
N_SUB_PER_KT = 4          # 4 subchunks of 125 per 500-wide k-tile
SUB = KT // N_SUB_PER_KT  # 125
N_SUB = CHUNK // SUB      # 16 subchunk ids per 2000-wide scan chunk
TAG_EPS = 2.0 ** -25
QCONST = 4.0
SIM_SCALE = 0.25          # host scales t_norm by this before casting

LAST_RESULTS = None    # per-core output dicts of the most recent run


def build_nc(dtype=DTYPE, kl=KL, with_index=True):
    """Build the single-core Bass program (SPMD across 8 cores).

    with_index=False (v3): drop the max_index pass and cand_i output --
    the host recovers indices by recomputing the <=8 winning 500-wide
    chunks per row (candidate slot -> chunk is static). Halves DVE work.
    """
    groups = [KT] * (kl // KT) if not with_index else groups_for(kl)
    n_grp = len(groups)
    ncand = 8 * n_grp
    mx = max(groups)
    # Bacc (not raw Bass): its compile() passes split multi-semaphore waits
    # (move_matmul_waits_to_ldweights / generate_event_semaphores) that the
    # walrus codegen's 1-wait-per-instruction limit requires.
    nc = bacc.Bacc()
    bankT = nc.declare_dram_parameter("bankT", [C, kl], dtype, isOutput=False)
    tT = nc.declare_dram_parameter("tT", [C, B], dtype, isOutput=False)
    cand_v = nc.declare_dram_parameter(
        "cand_v", [B, ncand], mybir.dt.float32, isOutput=True
    )
    cand_i = None
    if with_index:
        cand_i = nc.declare_dram_parameter(
            "cand_i", [B, ncand], mybir.dt.uint32, isOutput=True
        )

    bankT_r = bankT.rearrange("(c p) k -> p c k", p=128)  # [128, 4, kl]
    tT_r = tT.rearrange("(c p) b -> p c b", p=128)        # [128, 4, B]

    with tile.TileContext(nc) as tc:
        with (
            tc.tile_pool(name="const", bufs=1) as constp,
            # bufs=4: with the max_index pass gone the PE chain paces the
            # schedule, and 4-deep bank prefetch keeps it fed (model:
            # 67.5us vs 70.5us at bufs=3; saturates at 4).
            tc.tile_pool(name="bank", bufs=4) as bankp,
            tc.tile_pool(name="sim", bufs=2) as simp,
            tc.tile_pool(name="cand", bufs=1) as candp,
            tc.tile_pool(name="ps", bufs=8, space="PSUM") as psp,
        ):
            tw = constp.tile([128, 4, B], dtype)
            nc.sync.dma_start(tw[:], tT_r[:])

            vals = [
                candp.tile([128, n_grp, 8], mybir.dt.float32, tag=f"v{b}", name=f"vals{b}")
                for b in range(2)
            ]
            idxs = None
            if with_index:
                idxs = [
                    candp.tile([128, n_grp, 8], mybir.dt.uint32, tag=f"i{b}", name=f"idxs{b}")
                    for b in range(2)
                ]

            kt = 0
            for g, chunk in enumerate(groups):
                sims = [
                    simp.tile([128, mx], mybir.dt.float32, tag=f"s{b}", name=f"sim{b}")
                    for b in range(2)
                ]
                for j in range(chunk // KT):
                    bk = bankp.tile([128, 4, KT], dtype, tag="bank")
                    if kt == 0:
                        # split the first load per c-chunk so the first
                        # matmul starts after 1/4 of the transfer
                        # (model: 64.7us vs 67.5us)
                        for c in range(4):
                            nc.sync.dma_start(
                                bk[:, c, :], bankT_r[:, c, 0:KT]
                            )
                    else:
                        nc.sync.dma_start(
                            bk[:], bankT_r[:, :, kt * KT:(kt + 1) * KT]
                        )
                    for b in range(2):
                        ps = psp.tile([128, KT], mybir.dt.float32, tag="ps")
                        for c in range(4):
                            nc.tensor.matmul(
                                ps[:],
                                tw[:, c, b * 128:(b + 1) * 128],
                                bk[:, c, :],
                                start=(c == 0),
                                stop=(c == 3),
                            )
                        nc.scalar.copy(sims[b][:, j * KT:(j + 1) * KT], ps[:])
                    kt += 1
                for b in range(2):
                    nc.vector.max(vals[b][:, g, :], sims[b][:, 0:chunk])
                    if with_index:
                        nc.vector.max_index(
                            idxs[b][:, g, :], vals[b][:, g, :], sims[b][:, 0:chunk]
                        )

            for b in range(2):
                nc.sync.dma_start(cand_v[b * 128:(b + 1) * 128, :], vals[b][:])
                if with_index:
                    nc.sync.dma_start(cand_i[b * 128:(b + 1) * 128, :], idxs[b][:])

    return nc


def _make_consts():
    """Host-side constant rows for the v2 tag matmuls, bf16 [1, 3500].

    Layout: [0:128) ones (rank-1 stationary); [500:1000) +4.0;
    [1000:1500) -4.0; [1500+j*500 : 2000+j*500) tag row for kt%4 == j:
    id*2^-25 with id = ((j*500+n) // SUB) % 16. All exact in bf16.
    """
    c = np.zeros((1, 3500), np.float32)
    c[0, 0:128] = 1.0
    c[0, 500:1000] = QCONST
    c[0, 1000:1500] = -QCONST
    n = np.arange(KT)
    for j in range(4):
        ids = (j * KT + n) // SUB % N_SUB
        c[0, 1500 + j * 500:2000 + j * 500] = ids * TAG_EPS
    return c.astype(ml_dtypes.bfloat16)


def build_nc_v2(dtype=mybir.dt.bfloat16, kl=KL):
    """Tagged single-scan variant: one DVE max8 pass, no max_index."""
    assert dtype == mybir.dt.bfloat16
    n_grp = kl // CHUNK
    ncand = 8 * n_grp
    nc = bacc.Bacc()
    bankT = nc.declare_dram_parameter("bankT", [C, kl], dtype, isOutput=False)
    tT = nc.declare_dram_parameter("tT", [C, B], dtype, isOutput=False)
    consts = nc.declare_dram_parameter("consts", [1, 3500], dtype, isOutput=False)
    cand_v = nc.declare_dram_parameter(
        "cand_v", [B, ncand], mybir.dt.float32, isOutput=True
    )

    bankT_r = bankT.rearrange("(c p) k -> p c k", p=128)  # [128, 4, kl]
    tT_r = tT.rearrange("(c p) b -> p c b", p=128)        # [128, 4, B]

    with tile.TileContext(nc) as tc:
        with (
            tc.tile_pool(name="const", bufs=1) as constp,
            tc.tile_pool(name="bank", bufs=3) as bankp,
            tc.tile_pool(name="sim", bufs=2) as simp,
            tc.tile_pool(name="cand", bufs=1) as candp,
            tc.tile_pool(name="ps", bufs=8, space="PSUM") as psp,
        ):
            tw = constp.tile([128, 4, B], dtype)
            nc.sync.dma_start(tw[:], tT_r[:])
            cst = constp.tile([1, 3500], dtype)
            nc.sync.dma_start(cst[:], consts[:])
            ones_r = cst[0:1, 0:128]
            q_r = cst[0:1, 500:1000]
            nq_r = cst[0:1, 1000:1500]
            tag_r = [cst[0:1, 1500 + j * 500:2000 + j * 500] for j in range(4)]

            vals = [
                candp.tile([128, n_grp, 8], mybir.dt.float32,
                           tag=f"v{b}", name=f"vals{b}")
                for b in range(2)
            ]

            for g in range(n_grp):
                sims = [
                    simp.tile([128, CHUNK], mybir.dt.float32,
                              tag=f"s{b}", name=f"sim{b}")
                    for b in range(2)
                ]
                for j in range(GRP):
                    kt = g * GRP + j
                    bk = bankp.tile([128, 4, KT], dtype, tag="bank")
                    nc.sync.dma_start(
                        bk[:], bankT_r[:, :, kt * KT:(kt + 1) * KT]
                    )
                    for b in range(2):
                        ps = psp.tile([128, KT], mybir.dt.float32, tag="ps",
                                      name="ps")
                        for c in range(4):
                            nc.tensor.matmul(
                                ps[:],
                                tw[:, c, b * 128:(b + 1) * 128],
                                bk[:, c, :],
                                start=(c == 0), stop=False,
                            )
                        # quantize then tag: +4, -4, +id*2^-25 (in order)
                        nc.tensor.matmul(ps[:], ones_r, q_r,
                                         start=False, stop=False)
                        nc.tensor.matmul(ps[:], ones_r, nq_r,
                                         start=False, stop=False)
                        nc.tensor.matmul(ps[:], ones_r, tag_r[j % 4],
                                         start=False, stop=True)
                        nc.scalar.copy(sims[b][:, j * KT:(j + 1) * KT], ps[:])
                for b in range(2):
                    nc.vector.max(vals[b][:, g, :], sims[b][:])

            for b in range(2):
                nc.sync.dma_start(cand_v[b * 128:(b + 1) * 128, :], vals[b][:])

    return nc


# ---------------------------------------------------------------------------
# v4: fp8 DoubleRow + two-engine chunk scoring.
#
#   - Inputs cast to fp8e4 host-side with power-of-2 scales (bank x8,
#     normalized t x64): halves DMA bytes vs bf16 (16.4 -> 8.2 MB/core) and
#     the PE runs DoubleRow fp8 (2 contraction planes of 128 per matmul,
#     0.5 cyc/row): 2 matmuls per 500-wide tile per 128-row half.
#   - Chunk scoring splits across two engines reading PSUM directly (the
#     old ScalarE-evict + DVE-max8 pipeline cost ~46us on EACH engine):
#       * 13/32 tiles: DVE max8 straight from PSUM -> top-8 values.
#       * 19/32 tiles: ScalarE Exp activation in-place in PSUM with
#         accum_out -> sum(exp(0.5*psum)) = a log-sum-exp chunk score.
#     Both reduce to a per-(row, 500-chunk) proxy for the chunk max
#     (psum = 512*sim_n, so LSE exponent scale = 256; ln(score)/256 ~ max
#     ~ cand0/512), comparable across engines on the host.
#   - Host epilogue: rank all 256 chunk proxies per row, exactly recompute
#     the top-W windows in fp32 (device is only a candidate generator) and
#     take the global top-5 with reference tie-breaking.
#
# Selection margin validated offline on the fixed inputs: fp8 sim noise
# sigma=1.7e-3; worst true-top-5 chunk rank under either proxy = 10 (of
# 256), so W=16 windows cover all 256 rows with 6 ranks of margin.
# ---------------------------------------------------------------------------
V4_T = KL // KT            # 32 tiles of 500 bank rows per core
V4_S_BANK = 8.0            # bank fp8 scale (power of 2; avoids subnormals)
V4_S_T = 64.0              # normalized-t fp8 scale
V4_ACT_SCALE = 0.5         # Exp scale on psum; s_eff = 8*64*0.5 = 256
V4_SEFF = V4_S_BANK * V4_S_T * V4_ACT_SCALE
V4_N_DVE = 13              # tiles scored by DVE max8 (rest: ScalarE LSE)
V4_W = 16                  # host-recomputed candidate windows per row


def v4_dve_tiles():
    """13 DVE-scored tiles spread evenly over the 32 (Bresenham)."""
    return [t for t in range(V4_T)
            if (t + 1) * V4_N_DVE // V4_T > t * V4_N_DVE // V4_T]


def build_nc_v4():
    """fp8 DoubleRow + split ACT/DVE chunk scoring (see module comment)."""
    fp8 = mybir.dt.float8e4
    dve_tiles = set(v4_dve_tiles())
    n_dve = len(dve_tiles)
    n_act = V4_T - n_dve
    nc = bacc.Bacc()
    # [p, t, j, i, n] = bank8[t*500+n, j*256+i*128+p]: per partition each
    # tile's 2000 bytes are contiguous (2KB DMA runs, vs 1KB strided in v3)
    bank8 = nc.declare_dram_parameter(
        "bank8", [128, V4_T, 2, 2, KT], fp8, isOutput=False
    )
    # [p, h, j, i, m] = t8[h*128+m, j*256+i*128+p]
    tw8 = nc.declare_dram_parameter("tw8", [128, 2, 2, 2, 128], fp8,
                                    isOutput=False)
    cand_v = nc.declare_dram_parameter(
        "cand_v", [B, n_dve, 8], mybir.dt.float32, isOutput=True
    )
    scores = nc.declare_dram_parameter(
        "scores", [B, n_act], mybir.dt.float32, isOutput=True
    )

    with tile.TileContext(nc) as tc:
        with (
            tc.tile_pool(name="const", bufs=1) as constp,
            tc.tile_pool(name="bank", bufs=4) as bankp,
            tc.tile_pool(name="cand", bufs=1) as candp,
            tc.tile_pool(name="ps", bufs=8, space="PSUM") as psp,
        ):
            tw = constp.tile([128, 2, 2, 2, 128], fp8)
            nc.sync.dma_start(tw[:], tw8[:])

            cands = [
                candp.tile([128, n_dve, 8], mybir.dt.float32,
                           tag=f"c{h}", name=f"cands{h}")
                for h in range(2)
            ]
            scs = [
                candp.tile([128, n_act], mybir.dt.float32,
                           tag=f"s{h}", name=f"scores{h}")
                for h in range(2)
            ]

            d_idx = a_idx = 0
            for t in range(V4_T):
                bk = bankp.tile([128, 2, 2, KT], fp8, tag="bank")
                if t == 0:
                    # split the first load so the first matmul starts after
                    # a quarter of the transfer
                    for j in range(2):
                        for i in range(2):
                            nc.sync.dma_start(bk[:, j, i], bank8[:, 0, j, i])
                else:
                    nc.sync.dma_start(bk[:], bank8[:, t])
                for h in range(2):
                    ps = psp.tile([128, KT], mybir.dt.float32, tag="ps")
                    for j in range(2):
                        nc.tensor.matmul(
                            ps[:],
                            tw[:, h, j],        # [128, 2, 128] stationary
                            bk[:, j],           # [128, 2, 500] moving
                            start=(j == 0),
                            stop=(j == 1),
                            perf_mode=mybir.MatmulPerfMode.DoubleRow,
                        )
                    if t in dve_tiles:
                        nc.vector.max(cands[h][:, d_idx, :], ps[:])
                    else:
                        nc.scalar.activation(
                            ps[:], ps[:],
                            mybir.ActivationFunctionType.Exp,
                            scale=V4_ACT_SCALE,
                            accum_out=scs[h][:, a_idx:a_idx + 1],
                        )
                if t in dve_tiles:
                    d_idx += 1
                else:
                    a_idx += 1

            for h in range(2):
                nc.sync.dma_start(cand_v[h * 128:(h + 1) * 128, :], cands[h][:])
                nc.sync.dma_start(scores[h * 128:(h + 1) * 128, :], scs[h][:])

    return nc


def v4_pack_inputs(t, bank):
    """Host-side fp8 packing for v4. Returns (bank8 [8*128, ...], tw8)."""
    f8 = ml_dtypes.float8_e4m3
    t_n = t / np.maximum(np.linalg.norm(t, axis=1, keepdims=True), EPS)
    t8 = (t_n * V4_S_T).astype(f8)                      # [B, C]
    b8 = (bank * V4_S_BANK).astype(f8)                  # [K, C]
    # bank8[p, t, j, i, n] = b8[core*KL + t*KT + n, j*256 + i*128 + p]
    bank8 = (
        b8.reshape(N_CORES, V4_T, KT, 2, 2, 128)        # m, t, n, j, i, p
        .transpose(0, 5, 1, 3, 4, 2)                    # m, p, t, j, i, n
        .reshape(N_CORES * 128, V4_T, 2, 2, KT)
    )
    # tw8[p, h, j, i, m] = t8[h*128+m, j*256+i*128+p]
    tw8_1 = (
        t8.reshape(2, 128, 2, 2, 128)                   # h, m, j, i, p
        .transpose(4, 0, 2, 3, 1)                       # p, h, j, i, m
    )
    tw8 = np.concatenate([tw8_1] * N_CORES, axis=0)
    return np.ascontiguousarray(bank8), np.ascontiguousarray(tw8)


def _run_v4(exe, t, bank):
    """fp8 candidate-generator path: returns per-row global top-5 indices."""
    global LAST_RESULTS
    bank8, tw8 = v4_pack_inputs(t, bank)
    concat = {"bank8": bank8, "tw8": tw8}
    results = exe([concat[n] for n in exe.in_names])
    LAST_RESULTS = results

    dve_tiles = v4_dve_tiles()
    act_tiles = [t_ for t_ in range(V4_T) if t_ not in set(dve_tiles)]
    # per-chunk proxy for the chunk max, in normalized-sim units
    proxy = np.empty((B, N_CORES, V4_T), np.float32)
    for m, r in enumerate(results):
        proxy[:, m, dve_tiles] = r["cand_v"][:, :, 0] / (V4_S_BANK * V4_S_T)
        proxy[:, m, act_tiles] = np.log(
            np.maximum(r["scores"], 1e-30)) / V4_SEFF
    proxy = proxy.reshape(B, N_CORES * V4_T)

    sel = np.argpartition(-proxy, V4_W - 1, axis=1)[:, :V4_W]  # [B, W] chunks
    t_n = t / np.maximum(np.linalg.norm(t, axis=1, keepdims=True), EPS)
    span = np.arange(KT, dtype=np.int64)
    top5 = np.empty((B, TOPK), np.int64)
    for b in range(B):
        widx = (sel[b].astype(np.int64)[:, None] * KT + span[None, :]).ravel()
        wsims = bank[widx] @ t_n[b]                     # fp32 exact windows
        o = np.lexsort((widx, -wsims))
        top5[b] = widx[o[:TOPK]]
    return top5


# ---------------------------------------------------------------------------
# v5: v4 + wider units and DMA/LDW batching, from the v4 HW trace:
#   - DMA active was 41-47us for 8.45MB (32 per-tile DMAs serialized on one
#     HWDGE ring, ~0.6us fixed each).  v5 loads 1MB groups (8 DMAs) and
#     alternates the sync/scalar HWDGE rings so fixed costs overlap.
#   - PE active was 36.4us (128 LDWEIGHTS, one per matmul -- DoubleRow
#     disables fast-weight-load).  v5 sweeps each stationary across a group
#     of 2 super-tiles (4 matmuls back-to-back per LDW).
#   - ACT 33.8us vs DVE 19us was unbalanced, and per-instruction overhead
#     (~400 cyc) dominated 500-elem scans.  v5 scores 1000-wide units
#     ([128, 2, 500] strided PSUM APs over a 2-bank tile) with an 8/8
#     super-tile split: ~20us each.
# Chunk width 1000: worst true-top-5 chunk rank on the fixed inputs = 10 of
# 128 (either proxy), W=16 windows -> margin 6.
# ---------------------------------------------------------------------------
V5_CW = 1000               # scoring chunk width (super-tile)
V5_T = KL // V5_CW         # 16 super-tiles per core
V5_G = 2                   # super-tiles per stationary-sweep group
V5_NG = V5_T // V5_G       # 8 groups (1MB DMA each)
V5_W = 16                  # host-recomputed 1000-wide windows per row


def v5_act_supers():
    """8 of 16 super-tiles scored by ScalarE LSE (alternating)."""
    return [s for s in range(V5_T) if s % 2 == 0]


def build_nc_v5():
    fp8 = mybir.dt.float8e4
    act_set = set(v5_act_supers())
    n_act = len(act_set)
    n_dve = V5_T - n_act
    nc = bacc.Bacc()
    # [p, g, s, j, i, n] = b8[(g*2+s)*1000 + n, j*256 + i*128 + p]
    bank8 = nc.declare_dram_parameter(
        "bank8", [128, V5_NG, V5_G, 2, 2, V5_CW], fp8, isOutput=False
    )
    tw8 = nc.declare_dram_parameter("tw8", [128, 2, 2, 2, 128], fp8,
                                    isOutput=False)
    cand_v = nc.declare_dram_parameter(
        "cand_v", [B, n_dve, 8], mybir.dt.float32, isOutput=True
    )
    scores = nc.declare_dram_parameter(
        "scores", [B, n_act], mybir.dt.float32, isOutput=True
    )

    with tile.TileContext(nc) as tc:
        with (
            tc.tile_pool(name="const", bufs=1) as constp,
            tc.tile_pool(name="bank", bufs=3) as bankp,
            tc.tile_pool(name="cand", bufs=1) as candp,
            tc.tile_pool(name="ps", bufs=4, space="PSUM") as psp,
        ):
            tw = constp.tile([128, 2, 2, 2, 128], fp8)
            nc.sync.dma_start(tw[:], tw8[:])

            cands = [
                candp.tile([128, n_dve, 8], mybir.dt.float32,
                           tag=f"c{h}", name=f"cands{h}")
                for h in range(2)
            ]
            scs = [
                candp.tile([128, n_act], mybir.dt.float32,
                           tag=f"s{h}", name=f"scores{h}")
                for h in range(2)
            ]

            d_idx = a_idx = 0
            for g in range(V5_NG):
                bk = bankp.tile([128, V5_G, 2, 2, V5_CW], fp8, tag="bank")
                dma_eng = nc.sync if g % 2 == 0 else nc.scalar
                if g == 0:
                    # finer first loads so the first matmuls start early
                    for s in range(V5_G):
                        for j in range(2):
                            nc.sync.dma_start(bk[:, s, j], bank8[:, 0, s, j])
                else:
                    dma_eng.dma_start(bk[:], bank8[:, g])
                # 2-bank psum tile per (h, s); matmul writes 500-col halves
                pss = [[psp.tile([128, 2, 512], mybir.dt.float32, tag="ps",
                                 name="ps")
                        for s in range(V5_G)] for h in range(2)]
                # stationary-major sweep: one LDW serves 4 matmuls
                for j in range(2):
                    for h in range(2):
                        for s in range(V5_G):
                            for c in range(2):
                                nc.tensor.matmul(
                                    pss[h][s][:, c, 0:500],
                                    tw[:, h, j],
                                    bk[:, s, j, :, c * 500:(c + 1) * 500],
                                    start=(j == 0),
                                    stop=(j == 1),
                                    perf_mode=mybir.MatmulPerfMode.DoubleRow,
                                )
                for s in range(V5_G):
                    su = g * V5_G + s
                    for h in range(2):
                        view = pss[h][s][:, :, 0:500]
                        if su in act_set:
                            nc.scalar.activation(
                                view, view,
                                mybir.ActivationFunctionType.Exp,
                                scale=V4_ACT_SCALE,
                                accum_out=scs[h][:, a_idx:a_idx + 1],
                            )
                        else:
                            nc.vector.max(cands[h][:, d_idx, :], view)
                    if su in act_set:
                        a_idx += 1
                    else:
                        d_idx += 1

            for h in range(2):
                nc.sync.dma_start(cand_v[h * 128:(h + 1) * 128, :], cands[h][:])
                nc.sync.dma_start(scores[h * 128:(h + 1) * 128, :], scs[h][:])

    return nc


def v5_pack_inputs(t, bank):
    """Host-side fp8 packing for v5."""
    f8 = ml_dtypes.float8_e4m3
    t_n = t / np.maximum(np.linalg.norm(t, axis=1, keepdims=True), EPS)
    t8 = (t_n * V4_S_T).astype(f8)
    b8 = (bank * V4_S_BANK).astype(f8)
    bank8 = (
        b8.reshape(N_CORES, V5_NG, V5_G, V5_CW, 2, 2, 128)  # m,g,s,n,j,i,p
        .transpose(0, 6, 1, 2, 4, 5, 3)                     # m,p,g,s,j,i,n
        .reshape(N_CORES * 128, V5_NG, V5_G, 2, 2, V5_CW)
    )
    tw8_1 = (
        t8.reshape(2, 128, 2, 2, 128)                       # h,m,j,i,p
        .transpose(4, 0, 2, 3, 1)                           # p,h,j,i,m
    )
    tw8 = np.concatenate([tw8_1] * N_CORES, axis=0)
    return np.ascontiguousarray(bank8), np.ascontiguousarray(tw8)


def _run_v5(exe, t, bank):
    global LAST_RESULTS
    bank8, tw8 = v5_pack_inputs(t, bank)
    concat = {"bank8": bank8, "tw8": tw8}
    results = exe([concat[n] for n in exe.in_names])
    LAST_RESULTS = results
    return _v5_select(results, t, bank)


# ---------------------------------------------------------------------------
# v10: v9 + three-ring cold start.  Cold DMA rings sustain only ~100GB/s
# for the first ~10us (~1 super per ring per 5us), but the PE consumes a
# super per 1.71us.  v10 spreads the first 8 supers across sync, GpSimd
# AND the ACT ring (ACT's own work only starts once su0 is computed), with
# 6-deep prefetch, and sizes the HAM warm-up to the su0 latency.
# ---------------------------------------------------------------------------
V10_PREFETCH = 6
V10_WARM_MM = 18
V10_PAD_MM = 6


def v10_ring(su):
    """Ring per super, ordered by expected COLD arrival time: rings run
    ~100GB/s for their first ~10us, so each ring's k-th transfer lands at
    ~(k * 5us); program order must match arrival order or the PE stalls
    on a super whose ring is still busy."""
    # weighted by measured ring rates (sync/scalar HWDGE ~200GB/s,
    # GpSimd SWDGE ~115GB/s): period-4 pattern sync,gpsimd,scalar,sync
    early = {1: "sync", 2: "gpsimd", 3: "scalar"}
    if su in early:
        return early[su]
    return ("sync", "gpsimd", "scalar", "sync")[su % 4]


def build_nc_v10():
    fp8 = mybir.dt.float8e4
    nc = bacc.Bacc()
    bank8 = nc.declare_dram_parameter(
        "bank8", [128, V5_T, 2, 2, V5_CW], fp8, isOutput=False
    )
    tw8 = nc.declare_dram_parameter("tw8", [128, 2, 2, 2, 128], fp8,
                                    isOutput=False)
    outs_d = nc.declare_dram_parameter(
        "outs", [128, V5_T, 9], mybir.dt.float32, isOutput=True
    )

    with tile.TileContext(nc) as tc:
        with (
            tc.tile_pool(name="const", bufs=1) as constp,
            tc.tile_pool(name="bank", bufs=V10_PREFETCH + 1) as bankp,
            tc.tile_pool(name="cand", bufs=1) as candp,
            tc.tile_pool(name="ps", bufs=4, space="PSUM") as psp,
        ):
            tw = constp.tile([128, 2, 2, 2, 128], fp8)
            warm = constp.tile([128, 2, 256], fp8)
            nc.vector.memset(warm[:], 0.0)

            outs = candp.tile([128, V5_T, 9], mybir.dt.float32)

            bks = {}
            engs = {"sync": nc.sync, "gpsimd": nc.gpsimd, "scalar": nc.scalar}

            def issue_dma(su):
                bk = bankp.tile([128, 2, 2, V5_CW], fp8, tag="bank",
                                name="bk")
                if su == 0:
                    # split the cold first super across both HWDGE rings
                    nc.sync.dma_start(bk[:, 0], bank8[:, 0, 0])
                    nc.scalar.dma_start(tw[:], tw8[:])
                    nc.scalar.dma_start(bk[:, 1], bank8[:, 0, 1])
                else:
                    engs[v10_ring(su)].dma_start(bk[:], bank8[:, su])
                bks[su] = bk

            for su in range(V10_PREFETCH):
                issue_dma(su)

            wps = psp.tile([128, 2, 512], mybir.dt.float32, tag="ps",
                           name="wps")
            for _ in range(V10_WARM_MM):
                nc.tensor.matmul(
                    wps[:, 0, 0:256],
                    warm[:, :, 0:128],
                    warm[:],
                    start=True, stop=True,
                    perf_mode=mybir.MatmulPerfMode.DoubleRow,
                )

            for su in range(V5_T):
                if su + V10_PREFETCH < V5_T:
                    issue_dma(su + V10_PREFETCH)
                if su == 1:
                    # bridge the su0->su1 cold-arrival gap (~1.5us) so the
                    # HAM clock gate stays open
                    pps = psp.tile([128, 2, 512], mybir.dt.float32,
                                   tag="ps", name="pps")
                    for _ in range(V10_PAD_MM):
                        nc.tensor.matmul(
                            pps[:, 0, 0:256],
                            warm[:, :, 0:128],
                            warm[:],
                            start=True, stop=True,
                            perf_mode=mybir.MatmulPerfMode.DoubleRow,
                        )
                bk = bks.pop(su)
                pss = [psp.tile([128, 2, 512], mybir.dt.float32, tag="ps",
                                name="ps")
                       for h in range(2)]
                for h in range(2):
                    for j in range(2):
                        for c in range(2):
                            nc.tensor.matmul(
                                pss[h][:, c, 0:500],
                                tw[:, h, j],
                                bk[:, j, :, c * 500:(c + 1) * 500],
                                start=(j == 0),
                                stop=(j == 1),
                                perf_mode=mybir.MatmulPerfMode.DoubleRow,
                            )
                nc.scalar.activation(
                    pss[0][:, :, 0:500], pss[0][:, :, 0:500],
                    mybir.ActivationFunctionType.Exp,
                    scale=V4_ACT_SCALE,
                    accum_out=outs[:, su, 8:9],
                )
                nc.vector.max(outs[:, su, 0:8], pss[1][:, :, 0:500])

            nc.sync.dma_start(outs_d[:], outs[:])

    return nc


def _run_v10(exe, t, bank):
    global LAST_RESULTS
    bank8, tw8 = v6_pack_inputs(t, bank)
    concat = {"bank8": bank8, "tw8": tw8}
    results = exe([concat[n] for n in exe.in_names])
    LAST_RESULTS = results
    return _v7_select(results, t, bank)


# ---------------------------------------------------------------------------
# v9: v7 + HAM warm-up + split cold-start, from the v7/v8 HW traces:
#   - v7's remaining MM-phase excess (~5us over the 27.3us fp8 peak) was
#     HAM clock-gate throttle: the PE sits idle through the ~6us DMA ramp,
#     so the first ~40 matmuls issue at half clock.  v9 issues 24 dummy
#     DoubleRow matmuls on a zeroed SBUF tile during the ramp -- free
#     (PE idle anyway) -- so real matmuls start at full clock.
#   - cold-start split as v7b: su0's two j-planes on the sync + ACT rings
#     (tw first, 131KB), su1 on GpSimd -> first MM ~3us earlier than v7.
# ---------------------------------------------------------------------------
V9_WARM_MM = 18


def build_nc_v9():
    fp8 = mybir.dt.float8e4
    nc = bacc.Bacc()
    bank8 = nc.declare_dram_parameter(
        "bank8", [128, V5_T, 2, 2, V5_CW], fp8, isOutput=False
    )
    tw8 = nc.declare_dram_parameter("tw8", [128, 2, 2, 2, 128], fp8,
                                    isOutput=False)
    outs_d = nc.declare_dram_parameter(
        "outs", [128, V5_T, 9], mybir.dt.float32, isOutput=True
    )

    with tile.TileContext(nc) as tc:
        with (
            tc.tile_pool(name="const", bufs=1) as constp,
            tc.tile_pool(name="bank", bufs=V6_PREFETCH + 1) as bankp,
            tc.tile_pool(name="cand", bufs=1) as candp,
            tc.tile_pool(name="ps", bufs=4, space="PSUM") as psp,
        ):
            tw = constp.tile([128, 2, 2, 2, 128], fp8)
            warm = constp.tile([128, 2, 256], fp8)
            nc.vector.memset(warm[:], 0.0)

            outs = candp.tile([128, V5_T, 9], mybir.dt.float32)

            bks = {}

            def issue_dma(su):
                bk = bankp.tile([128, 2, 2, V5_CW], fp8, tag="bank",
                                name="bk")
                if su == 0:
                    nc.sync.dma_start(bk[:, 0], bank8[:, 0, 0])
                    nc.scalar.dma_start(tw[:], tw8[:])
                    nc.scalar.dma_start(bk[:, 1], bank8[:, 0, 1])
                else:
                    eng = nc.sync if su % 2 == 0 else nc.gpsimd
                    eng.dma_start(bk[:], bank8[:, su])
                bks[su] = bk

            for su in range(V6_PREFETCH):
                issue_dma(su)

            # HAM warm-up: keep the PE busy through the DMA ramp so the
            # clock gate opens before the first real matmul
            wps = psp.tile([128, 2, 512], mybir.dt.float32, tag="ps",
                           name="wps")
            for _ in range(V9_WARM_MM):
                nc.tensor.matmul(
                    wps[:, 0, 0:256],
                    warm[:, :, 0:128],
                    warm[:],
                    start=True, stop=True,
                    perf_mode=mybir.MatmulPerfMode.DoubleRow,
                )

            for su in range(V5_T):
                if su + V6_PREFETCH < V5_T:
                    issue_dma(su + V6_PREFETCH)
                bk = bks.pop(su)
                pss = [psp.tile([128, 2, 512], mybir.dt.float32, tag="ps",
                                name="ps")
                       for h in range(2)]
                for h in range(2):
                    for j in range(2):
                        for c in range(2):
                            nc.tensor.matmul(
                                pss[h][:, c, 0:500],
                                tw[:, h, j],
                                bk[:, j, :, c * 500:(c + 1) * 500],
                                start=(j == 0),
                                stop=(j == 1),
                                perf_mode=mybir.MatmulPerfMode.DoubleRow,
                            )
                nc.scalar.activation(
                    pss[0][:, :, 0:500], pss[0][:, :, 0:500],
                    mybir.ActivationFunctionType.Exp,
                    scale=V4_ACT_SCALE,
                    accum_out=outs[:, su, 8:9],
                )
                nc.vector.max(outs[:, su, 0:8], pss[1][:, :, 0:500])

            nc.sync.dma_start(outs_d[:], outs[:])

    return nc


def _run_v9(exe, t, bank):
    global LAST_RESULTS
    bank8, tw8 = v6_pack_inputs(t, bank)
    concat = {"bank8": bank8, "tw8": tw8}
    results = exe([concat[n] for n in exe.in_names])
    LAST_RESULTS = results
    return _v7_select(results, t, bank)


# ---------------------------------------------------------------------------
# v8: v7 + pair-DMAs and h-major matmuls, from the v7 HW trace:
#   - Per-super 524KB DMAs (4000B per-partition runs) sustained only
#     ~169GB/s per ring; v5's 1MB pair transfers with 8000B runs hit
#     ~318GB/s.  v8 reloads the pair layout: one DMA per 2 super-tiles,
#     alternating sync/GpSimd rings (first pair split su0->sync,
#     su1->ACT-ring so the PE starts early), consumed per super.
#   - v7 issued matmuls j-major, so a super's h0 PSUM tile was only
#     complete ~1.3us into its 1.71us MM phase and the 4-tile PSUM pool
#     left PE ~0.3us/super of recycle slack.  v8 issues h-major (h0's two
#     j-planes first): h0's consumer starts at 0.86us, roughly 1.2us of
#     slack per tile.
# ---------------------------------------------------------------------------


def build_nc_v8():
    fp8 = mybir.dt.float8e4
    nc = bacc.Bacc()
    # [p, g, s, j, i, n] = b8[(g*2+s)*1000 + n, j*256 + i*128 + p]
    bank8 = nc.declare_dram_parameter(
        "bank8", [128, V5_NG, V5_G, 2, 2, V5_CW], fp8, isOutput=False
    )
    tw8 = nc.declare_dram_parameter("tw8", [128, 2, 2, 2, 128], fp8,
                                    isOutput=False)
    outs_d = nc.declare_dram_parameter(
        "outs", [128, V5_T, 9], mybir.dt.float32, isOutput=True
    )

    with tile.TileContext(nc) as tc:
        with (
            tc.tile_pool(name="const", bufs=1) as constp,
            tc.tile_pool(name="bank", bufs=3) as bankp,
            tc.tile_pool(name="cand", bufs=1) as candp,
            tc.tile_pool(name="ps", bufs=4, space="PSUM") as psp,
        ):
            tw = constp.tile([128, 2, 2, 2, 128], fp8)

            outs = candp.tile([128, V5_T, 9], mybir.dt.float32)

            bkp = {}

            def issue_pair(g):
                bk = bankp.tile([128, V5_G, 2, 2, V5_CW], fp8, tag="bank",
                                name="bk")
                if g == 0:
                    # cold start: su0/su1 split across the sync and GpSimd
                    # rings, tw (131KB) alone on the ACT ring
                    nc.sync.dma_start(bk[:, 0], bank8[:, 0, 0])
                    nc.scalar.dma_start(tw[:], tw8[:])
                    nc.gpsimd.dma_start(bk[:, 1], bank8[:, 0, 1])
                else:
                    eng = nc.sync if g % 2 == 1 else nc.gpsimd
                    eng.dma_start(bk[:], bank8[:, g])
                bkp[g] = bk

            issue_pair(0)
            issue_pair(1)

            for su in range(V5_T):
                g, s = divmod(su, V5_G)
                if s == 0 and g + 2 < V5_NG:
                    issue_pair(g + 2)
                bk = bkp[g]
                pss = [psp.tile([128, 2, 512], mybir.dt.float32, tag="ps",
                                name="ps")
                       for h in range(2)]
                for h in range(2):
                    for j in range(2):
                        for c in range(2):
                            nc.tensor.matmul(
                                pss[h][:, c, 0:500],
                                tw[:, h, j],
                                bk[:, s, j, :, c * 500:(c + 1) * 500],
                                start=(j == 0),
                                stop=(j == 1),
                                perf_mode=mybir.MatmulPerfMode.DoubleRow,
                            )
                nc.scalar.activation(
                    pss[0][:, :, 0:500], pss[0][:, :, 0:500],
                    mybir.ActivationFunctionType.Exp,
                    scale=V4_ACT_SCALE,
                    accum_out=outs[:, su, 8:9],
                )
                nc.vector.max(outs[:, su, 0:8], pss[1][:, :, 0:500])

            nc.sync.dma_start(outs_d[:], outs[:])

    return nc


def _run_v8(exe, t, bank):
    global LAST_RESULTS
    bank8, tw8 = v5_pack_inputs(t, bank)
    concat = {"bank8": bank8, "tw8": tw8}
    results = exe([concat[n] for n in exe.in_names])
    LAST_RESULTS = results
    return _v7_select(results, t, bank)


def _v7_select(results, t, bank):
    """Shared v7/v8 host epilogue (packed [128, 16, 9] outs)."""
    proxy = np.empty((B, N_CORES, V5_T), np.float32)
    for m, r in enumerate(results):
        o = r["outs"]                                     # [128, 16, 9]
        proxy[:128, m, :] = np.log(
            np.maximum(o[:, :, 8], 1e-30)) / V4_SEFF
        proxy[128:, m, :] = o[:, :, 0] / (V4_S_BANK * V4_S_T)
    proxy = proxy.reshape(B, N_CORES * V5_T)

    sel = np.argpartition(-proxy, V5_W - 1, axis=1)[:, :V5_W]
    t_n = t / np.maximum(np.linalg.norm(t, axis=1, keepdims=True), EPS)
    span = np.arange(V5_CW, dtype=np.int64)
    top5 = np.empty((B, TOPK), np.int64)
    for b in range(B):
        widx = (sel[b].astype(np.int64)[:, None] * V5_CW
                + span[None, :]).ravel()
        wsims = bank[widx] @ t_n[b]
        o = np.lexsort((widx, -wsims))
        top5[b] = widx[o[:TOPK]]
    return top5


# ---------------------------------------------------------------------------
# v7: v6 + trigger/tail/interleave fixes, from the v6 HW trace:
#   - Each HWDGE dma_start costs ~600ns on its issuing engine: the 8-way
#     split first load burned ~4us of ramp, and 4 output DMAs ~2.4us of
#     tail.  v7 issues one DMA per super-tile (even: sync ring, odd: the
#     otherwise-idle GpSimd SWDGE ring, keeping the ACT queue clean) and
#     packs all outputs into one [128, 16, 9] tile (single DMA).
#   - v6 ran both of a super's consumers serially on one engine (2.7us)
#     while the other idled.  v7 sends half h=0 to ScalarE-LSE and h=1 to
#     DVE-max8 for EVERY super: the two run in parallel (~1.4us/super).
#     Each B-row is scored by a single proxy type for all its chunks
#     (rows 0-127: LSE, rows 128-255: max8) -- both validated offline.
# ---------------------------------------------------------------------------
V7_PREFETCH = 4


def build_nc_v7():
    fp8 = mybir.dt.float8e4
    nc = bacc.Bacc()
    bank8 = nc.declare_dram_parameter(
        "bank8", [128, V5_T, 2, 2, V5_CW], fp8, isOutput=False
    )
    tw8 = nc.declare_dram_parameter("tw8", [128, 2, 2, 2, 128], fp8,
                                    isOutput=False)
    # [p, su, 0:8] = top-8 of super su for row 128+p (DVE half);
    # [p, su, 8]   = sum(exp(0.5*psum)) of super su for row p (ACT half)
    outs_d = nc.declare_dram_parameter(
        "outs", [128, V5_T, 9], mybir.dt.float32, isOutput=True
    )

    with tile.TileContext(nc) as tc:
        with (
            tc.tile_pool(name="const", bufs=1) as constp,
            tc.tile_pool(name="bank", bufs=V7_PREFETCH + 1) as bankp,
            tc.tile_pool(name="cand", bufs=1) as candp,
            tc.tile_pool(name="ps", bufs=4, space="PSUM") as psp,
        ):
            tw = constp.tile([128, 2, 2, 2, 128], fp8)
            warm = constp.tile([128, 2, 256], fp8)
            nc.vector.memset(warm[:], 0.0)

            outs = candp.tile([128, V5_T, 9], mybir.dt.float32)

            bks = {}

            def issue_dma(su):
                bk = bankp.tile([128, 2, 2, V5_CW], fp8, tag="bank",
                                name="bk")
                if su <= 1:
                    # cold-start: halve the first two supers across the two
                    # HWDGE rings; tw (131KB) rides the otherwise-idle ACT
                    # ring.  Both su1 halves land ~1.5us before a whole-su1
                    # sync transfer would.
                    if su == 0:
                        nc.sync.dma_start(bk[:, 0], bank8[:, 0, 0])
                        nc.scalar.dma_start(tw[:], tw8[:])
                        nc.scalar.dma_start(bk[:, 1], bank8[:, 0, 1])
                    else:
                        nc.sync.dma_start(bk[:, 0], bank8[:, 1, 0])
                        nc.scalar.dma_start(bk[:, 1], bank8[:, 1, 1])
                else:
                    # 11/5 ring split: the SWDGE (GpSimd) ring sustains
                    # only ~115GB/s vs sync HWDGE's ~180-230 warm, and
                    # carrying half the bytes it pinned the last super's
                    # arrival (and thus the last matmul) at ~51.5k
                    # su1 must beat GpSimd's slow (~6us) first SWDGE
                    # transfer: sync's 2nd lands ~2us earlier
                    eng = (nc.gpsimd if su >= 2 and (su - 2) % 3 == 0
                           else nc.sync)
                    eng.dma_start(bk[:], bank8[:, su])
                bks[su] = bk

            for su in range(V7_PREFETCH):
                issue_dma(su)

            # start-only HAM warm-up: dummy DoubleRow MMs on the zeroed
            # tile keep the clock gate open through the ~3us DMA ramp so
            # su0's real matmuls issue at the full 213ns rate.  (A pad
            # block BETWEEN supers stalls on scheduler semaphores -- v16.)
            wps = psp.tile([128, 2, 512], mybir.dt.float32, tag="ps",
                           name="wps")
            for _ in range(14):
                nc.tensor.matmul(
                    wps[:, 0, 0:256],
                    warm[:, :, 0:128],
                    warm[:],
                    start=True, stop=True,
                    perf_mode=mybir.MatmulPerfMode.DoubleRow,
                )

            for su in range(V5_T):
                if su + V7_PREFETCH < V5_T:
                    issue_dma(su + V7_PREFETCH)
                bk = bks.pop(su)
                pss = [psp.tile([128, 2, 512], mybir.dt.float32, tag="ps",
                                name="ps")
                       for h in range(2)]
                for j in range(2):
                    for h in range(2):
                        for c in range(2):
                            nc.tensor.matmul(
                                pss[h][:, c, 0:500],
                                tw[:, h, j],
                                bk[:, j, :, c * 500:(c + 1) * 500],
                                start=(j == 0),
                                stop=(j == 1),
                                perf_mode=mybir.MatmulPerfMode.DoubleRow,
                            )
                nc.scalar.activation(
                    pss[0][:, :, 0:500], pss[0][:, :, 0:500],
                    mybir.ActivationFunctionType.Exp,
                    scale=V4_ACT_SCALE,
                    accum_out=outs[:, su, 8:9],
                )
                nc.vector.max(outs[:, su, 0:8], pss[1][:, :, 0:500])

            nc.sync.dma_start(outs_d[:], outs[:])

    return nc


def _run_v7(exe, t, bank):
    global LAST_RESULTS
    bank8, tw8 = v6_pack_inputs(t, bank)
    concat = {"bank8": bank8, "tw8": tw8}
    results = exe([concat[n] for n in exe.in_names])
    LAST_RESULTS = results

    # proxy per (row, core, super): rows 0-127 from the LSE scores,
    # rows 128-255 from the max8 values -- both ~ chunk max in sim units
    proxy = np.empty((B, N_CORES, V5_T), np.float32)
    for m, r in enumerate(results):
        o = r["outs"]                                     # [128, 16, 9]
        proxy[:128, m, :] = np.log(
            np.maximum(o[:, :, 8], 1e-30)) / V4_SEFF
        proxy[128:, m, :] = o[:, :, 0] / (V4_S_BANK * V4_S_T)
    proxy = proxy.reshape(B, N_CORES * V5_T)

    sel = np.argpartition(-proxy, V5_W - 1, axis=1)[:, :V5_W]
    t_n = t / np.maximum(np.linalg.norm(t, axis=1, keepdims=True), EPS)
    span = np.arange(V5_CW, dtype=np.int64)
    top5 = np.empty((B, TOPK), np.int64)
    for b in range(B):
        widx = (sel[b].astype(np.int64)[:, None] * V5_CW
                + span[None, :]).ravel()
        wsims = bank[widx] @ t_n[b]
        o = np.lexsort((widx, -wsims))
        top5[b] = widx[o[:TOPK]]
    return top5


# ---------------------------------------------------------------------------
# v6: v5 + deep software pipelining, from the v5 HW trace:
#   - Warm PE issues a DoubleRow MM every 210ns (fp8 peak, 157 TF/s; 27us
#     for all 128), but ~half the kernel ran HAM-throttled (420ns/MM)
#     because of 4-5us PE idle gaps at group boundaries: the 4-psum-tile
#     groups consumed ALL of PSUM (no cross-group overlap), and odd-group
#     DMA triggers sat behind the previous group's ACT instructions in the
#     ACT queue.
#   - v6: one super-tile per step (2 psum tiles -> two steps in flight),
#     per-super 524KB DMAs issued 3 steps ahead, alternating the sync/ACT
#     HWDGE rings, with the trigger emitted BEFORE the step's consumers.
# ---------------------------------------------------------------------------
V6_PREFETCH = 3


def build_nc_v6():
    fp8 = mybir.dt.float8e4
    act_set = set(v5_act_supers())
    n_act = len(act_set)
    n_dve = V5_T - n_act
    nc = bacc.Bacc()
    # [p, s, j, i, n] = b8[s*1000 + n, j*256 + i*128 + p]
    bank8 = nc.declare_dram_parameter(
        "bank8", [128, V5_T, 2, 2, V5_CW], fp8, isOutput=False
    )
    tw8 = nc.declare_dram_parameter("tw8", [128, 2, 2, 2, 128], fp8,
                                    isOutput=False)
    cand_v = nc.declare_dram_parameter(
        "cand_v", [B, n_dve, 8], mybir.dt.float32, isOutput=True
    )
    scores = nc.declare_dram_parameter(
        "scores", [B, n_act], mybir.dt.float32, isOutput=True
    )

    with tile.TileContext(nc) as tc:
        with (
            tc.tile_pool(name="const", bufs=1) as constp,
            tc.tile_pool(name="bank", bufs=V6_PREFETCH + 1) as bankp,
            tc.tile_pool(name="cand", bufs=1) as candp,
            tc.tile_pool(name="ps", bufs=4, space="PSUM") as psp,
        ):
            tw = constp.tile([128, 2, 2, 2, 128], fp8)
            nc.sync.dma_start(tw[:], tw8[:])

            cands = [
                candp.tile([128, n_dve, 8], mybir.dt.float32,
                           tag=f"c{h}", name=f"cands{h}")
                for h in range(2)
            ]
            scs = [
                candp.tile([128, n_act], mybir.dt.float32,
                           tag=f"s{h}", name=f"scores{h}")
                for h in range(2)
            ]

            bks = {}

            def issue_dma(su):
                bk = bankp.tile([128, 2, 2, V5_CW], fp8, tag="bank",
                                name="bk")
                if su == 0:
                    # fine-grained first load: first matmul starts after 1/4
                    for j in range(2):
                        for i in range(2):
                            nc.sync.dma_start(bk[:, j, i], bank8[:, 0, j, i])
                else:
                    eng = nc.sync if su % 2 == 0 else nc.scalar
                    eng.dma_start(bk[:], bank8[:, su])
                bks[su] = bk

            for su in range(V6_PREFETCH):
                issue_dma(su)

            d_idx = a_idx = 0
            for su in range(V5_T):
                if su + V6_PREFETCH < V5_T:
                    issue_dma(su + V6_PREFETCH)
                bk = bks.pop(su)
                pss = [psp.tile([128, 2, 512], mybir.dt.float32, tag="ps",
                                name="ps")
                       for h in range(2)]
                for j in range(2):
                    for h in range(2):
                        for c in range(2):
                            nc.tensor.matmul(
                                pss[h][:, c, 0:500],
                                tw[:, h, j],
                                bk[:, j, :, c * 500:(c + 1) * 500],
                                start=(j == 0),
                                stop=(j == 1),
                                perf_mode=mybir.MatmulPerfMode.DoubleRow,
                            )
                for h in range(2):
                    view = pss[h][:, :, 0:500]
                    if su in act_set:
                        nc.scalar.activation(
                            view, view,
                            mybir.ActivationFunctionType.Exp,
                            scale=V4_ACT_SCALE,
                            accum_out=scs[h][:, a_idx:a_idx + 1],
                        )
                    else:
                        nc.vector.max(cands[h][:, d_idx, :], view)
                if su in act_set:
                    a_idx += 1
                else:
                    d_idx += 1

            for h in range(2):
                nc.sync.dma_start(cand_v[h * 128:(h + 1) * 128, :], cands[h][:])
                nc.sync.dma_start(scores[h * 128:(h + 1) * 128, :], scs[h][:])

    return nc


def v6_pack_inputs(t, bank):
    """Host-side fp8 packing for v6 (per-super-tile layout)."""
    f8 = ml_dtypes.float8_e4m3
    t_n = t / np.maximum(np.linalg.norm(t, axis=1, keepdims=True), EPS)
    t8 = (t_n * V4_S_T).astype(f8)
    b8 = (bank * V4_S_BANK).astype(f8)
    bank8 = (
        b8.reshape(N_CORES, V5_T, V5_CW, 2, 2, 128)     # m,s,n,j,i,p
        .transpose(0, 5, 1, 3, 4, 2)                    # m,p,s,j,i,n
        .reshape(N_CORES * 128, V5_T, 2, 2, V5_CW)
    )
    tw8_1 = (
        t8.reshape(2, 128, 2, 2, 128)                   # h,m,j,i,p
        .transpose(4, 0, 2, 3, 1)                       # p,h,j,i,m
    )
    tw8 = np.concatenate([tw8_1] * N_CORES, axis=0)
    return np.ascontiguousarray(bank8), np.ascontiguousarray(tw8)


def _run_v6(exe, t, bank):
    global LAST_RESULTS
    bank8, tw8 = v6_pack_inputs(t, bank)
    concat = {"bank8": bank8, "tw8": tw8}
    results = exe([concat[n] for n in exe.in_names])
    LAST_RESULTS = results
    return _v5_select(results, t, bank)


def _v5_select(results, t, bank):
    """Shared v5/v6 host epilogue: proxies -> windows -> exact top-5."""
    act_supers = v5_act_supers()
    dve_supers = [s for s in range(V5_T) if s not in set(act_supers)]
    proxy = np.empty((B, N_CORES, V5_T), np.float32)
    for m, r in enumerate(results):
        proxy[:, m, dve_supers] = r["cand_v"][:, :, 0] / (V4_S_BANK * V4_S_T)
        proxy[:, m, act_supers] = np.log(
            np.maximum(r["scores"], 1e-30)) / V4_SEFF
    proxy = proxy.reshape(B, N_CORES * V5_T)

    sel = np.argpartition(-proxy, V5_W - 1, axis=1)[:, :V5_W]
    t_n = t / np.maximum(np.linalg.norm(t, axis=1, keepdims=True), EPS)
    span = np.arange(V5_CW, dtype=np.int64)
    top5 = np.empty((B, TOPK), np.int64)
    for b in range(B):
        widx = (sel[b].astype(np.int64)[:, None] * V5_CW
                + span[None, :]).ravel()
        wsims = bank[widx] @ t_n[b]
        o = np.lexsort((widx, -wsims))
        top5[b] = widx[o[:TOPK]]
    return top5


# "v1": two DVE scans per chunk (max8 + max_index) -- simplest, and the
#       faster schedule under the TRN2 instruction cost model (87.8us vs
#       109.6us predicted per core; DVE-bound).
# "v2": tagged single-scan -- one DVE max8 pass; the PE quantizes sims
#       in-PSUM (+4/-4 rank-1s) and adds a sub-quantum subchunk tag that
#       the host decodes, trading DVE time for PE time. Better if real
#       silicon streams bf16 matmuls near the documented 131ns/MM rate.
# "v3": v1's matmul+max8 pipeline with NO max_index pass at all -- the
#       candidate slot already identifies the 500-wide chunk, so the host
#       recomputes the <=8 best chunks per row (~1 GFLOP) to recover exact
#       indices. Halves DVE work; model-predicted 70.5us vs 84.5us (v1).
# "v4": fp8 DoubleRow matmuls + ACT/DVE split chunk scoring from PSUM --
#       see the block comment above build_nc_v4.
# "v5": v4 + 1MB dual-ring DMAs, stationary-swept matmul groups, 1000-wide
#       scoring units -- see the block comment above build_nc_v5.
# v1-v3 validated on the fixed inputs (HW): v1 loss rel err 4.9e-5,
# v2 5.3e-6, v3 4.9e-5; purity exact in all.  v4 rel err 0.0 (HW).
MODE = "v7"

_NC_CACHE = {}


def _get_nc():
    key = (MODE, DTYPE)
    if key not in _NC_CACHE:
        if MODE == "v10":
            nc = build_nc_v10()
        elif MODE == "v9":
            nc = build_nc_v9()
        elif MODE == "v8":
            nc = build_nc_v8()
        elif MODE == "v7":
            nc = build_nc_v7()
        elif MODE == "v6":
            nc = build_nc_v6()
        elif MODE == "v5":
            nc = build_nc_v5()
        elif MODE == "v4":
            nc = build_nc_v4()
        elif MODE == "v2":
            nc = build_nc_v2()
        elif MODE == "v3":
            nc = build_nc(DTYPE, with_index=False)
        else:
            nc = build_nc(DTYPE)
        nc.finalize()
        _NC_CACHE[key] = nc
    return _NC_CACHE[key]


class _SpmdExec:
    """Cached jitted shard_map over the bass_exec custom call.

    Mirrors bass2jax.run_bass_via_pjrt's multi-core path but builds the
    jitted executable once, so repeated calls skip retrace/recompile.
    """

    def __init__(self, nc):
        bass2jax.install_neuronx_cc_hook()
        part_name = (
            nc.partition_id_tensor.name if nc.partition_id_tensor else None
        )
        in_names, out_names, out_avals = [], [], []
        for alloc in nc.m.functions[0].allocations:
            if not isinstance(alloc, mybir.MemoryLocationSet):
                continue
            name = alloc.memorylocations[0].name
            if alloc.kind == "ExternalInput":
                if name != part_name:
                    in_names.append(name)
            elif alloc.kind == "ExternalOutput":
                out_names.append(name)
                out_avals.append(
                    jax.core.ShapedArray(
                        tuple(alloc.tensor_shape), mybir.dt.np(alloc.dtype)
                    )
                )
        self.in_names = list(in_names)
        self.out_names = out_names
        self.out_avals = out_avals
        n_params = len(in_names)
        n_outs = len(out_names)
        bind_names = in_names + out_names
        if part_name is not None:
            bind_names = bind_names + [part_name]
        bind_names = tuple(bind_names)

        def _body(*args):
            operands = list(args)
            if part_name is not None:
                operands.append(bass2jax.partition_id_tensor())
            outs = bass2jax._bass_exec_p.bind(
                *operands,
                out_avals=tuple(out_avals),
                in_names=bind_names,
                out_names=tuple(out_names),
                lowering_input_output_aliases=(),
                sim_require_finite=True,
                sim_require_nnan=True,
                nc=nc,
            )
            return tuple(outs)

        devices = jax.devices()[:N_CORES]
        self.mesh = Mesh(np.asarray(devices), ("core",))
        in_specs = (PartitionSpec("core"),) * (n_params + n_outs)
        out_specs = (PartitionSpec("core"),) * n_outs
        self.fn = jax.jit(
            shard_map(
                _body,
                mesh=self.mesh,
                in_specs=in_specs,
                out_specs=out_specs,
                check_rep=False,
            ),
            donate_argnums=tuple(range(n_params, n_params + n_outs)),
            keep_unused=True,
        )

    def zero_outs(self):
        return [
            np.zeros((N_CORES * a.shape[0], *a.shape[1:]), a.dtype)
            for a in self.out_avals
        ]

    def __call__(self, concat_inputs):
        """concat_inputs: list matching in_names, each (N_CORES*dim0, ...)."""
        out_arrs = self.fn(*concat_inputs, *self.zero_outs())
        return [
            {
                name: np.asarray(out_arrs[i]).reshape(
                    N_CORES, *self.out_avals[i].shape
                )[c]
                for i, name in enumerate(self.out_names)
            }
            for c in range(N_CORES)
        ]


_EXEC_CACHE = {}


def _get_exec():
    key = (MODE, DTYPE)
    if key not in _EXEC_CACHE:
        _EXEC_CACHE[key] = _SpmdExec(_get_nc())
    return _EXEC_CACHE[key]


def _np_dtype(dtype):
    return ml_dtypes.bfloat16 if dtype == mybir.dt.bfloat16 else np.float32


def _run_v1(exe, bank_sh, t, tT):
    """max8 + max_index path: returns per-row global top-5 indices."""
    global LAST_RESULTS
    np_dt = _np_dtype(DTYPE)
    tT_c = tT.astype(np_dt)
    concat = {
        "bankT": bank_sh,
        "tT": np.concatenate([tT_c] * N_CORES, axis=0),
    }
    results = exe([concat[n] for n in exe.in_names])
    LAST_RESULTS = results

    vals = np.stack([r["cand_v"] for r in results], axis=1)
    idx_l = np.stack(
        [r["cand_i"].astype(np.int64) for r in results], axis=1
    )
    groups = groups_for(KL)
    gbase = np.concatenate([[0], np.cumsum(groups)[:-1]]).astype(np.int64)
    base = (
        np.arange(N_CORES, dtype=np.int64)[None, :, None] * KL
        + np.repeat(gbase, 8)[None, None, :]
    )
    gidx = (idx_l + base).reshape(B, -1)            # global indices
    vals = vals.reshape(B, -1)                      # raw sim_t

    # Emulate the reference's comparison domain: fp32 dist_t with per-row
    # 1/||t_b|| folded back in; ties break toward the lowest global index.
    inv_t = 1.0 / np.maximum(np.linalg.norm(t, axis=1), EPS)   # [B]
    dist32 = (2.0 - 2.0 * vals * inv_t[:, None]).astype(np.float32)
    top5 = np.empty((B, TOPK), np.int64)
    for b in range(B):
        order = np.lexsort((gidx[b], dist32[b]))
        top5[b] = gidx[b][order[:TOPK]]
    return top5


N_WINDOWS = 10  # per-row candidate windows recomputed exactly on the host


def _run_v2(exe, bank_sh, t, bank):
    """Tagged single-scan path: returns per-row global top-5 indices."""
    global LAST_RESULTS
    bf = ml_dtypes.bfloat16
    t_n = t / np.maximum(np.linalg.norm(t, axis=1, keepdims=True), EPS)
    tw = np.ascontiguousarray((t_n * SIM_SCALE).T).astype(bf)   # [C, B]
    consts = _make_consts()
    concat = {
        "bankT": bank_sh,
        "tT": np.concatenate([tw] * N_CORES, axis=0),
        "consts": np.concatenate([consts] * N_CORES, axis=0),
    }
    results = exe([concat[n] for n in exe.in_names])
    LAST_RESULTS = results

    # packed candidates [B, N_CORES, NCAND]
    packed = np.stack([r["cand_v"] for r in results], axis=1)
    pk = packed.reshape(B, -1).astype(np.float64)    # [B, 512]
    # packed = q(sim) + id*2^-25 with q a multiple of 2^-21 (positive sims)
    y = np.round(pk / TAG_EPS).astype(np.int64)      # exact integer
    dec_id = np.mod(y, N_SUB)
    qsim = pk - dec_id * TAG_EPS                     # quantized scaled sim
    # window start (global bank row) per candidate
    cores = np.repeat(np.arange(N_CORES, dtype=np.int64), NCAND)[None, :]
    groups = np.tile(
        np.repeat(np.arange(N_GRP, dtype=np.int64), 8), N_CORES
    )[None, :]
    wstart = cores * KL + groups * CHUNK + dec_id * SUB   # [B, 512]

    # top-N_WINDOWS candidates per row by qsim; recompute those 125-wide
    # windows exactly (fp32 over the bf16-cast operands, matching the
    # device's computation up to summation order) and take the exact top-5.
    order = np.argsort(-qsim, axis=1, kind="stable")[:, :N_WINDOWS]
    sel_start = np.take_along_axis(wstart, order, axis=1)     # [B, W]

    bank_bf = bank.astype(bf).astype(np.float32)              # [K, C]
    t_bf = (t_n * SIM_SCALE).astype(bf).astype(np.float32)    # [B, C]
    flat_idx = (sel_start[:, :, None] +
                np.arange(SUB, dtype=np.int64)[None, None, :])  # [B, W, SUB]
    rows = bank_bf[flat_idx.reshape(-1)].reshape(B, N_WINDOWS * SUB, C)
    wsims = np.einsum("bkc,bc->bk", rows, t_bf)               # [B, W*SUB]
    widx = flat_idx.reshape(B, -1)                            # [B, W*SUB]

    top5 = np.empty((B, TOPK), np.int64)
    for b in range(B):
        # windows may overlap -> dedupe indices, keep exact values
        o = np.lexsort((widx[b], -wsims[b]))
        seen, picks = set(), []
        for i in o:
            gi = widx[b, i]
            if gi in seen:
                continue
            seen.add(gi)
            picks.append(gi)
            if len(picks) == TOPK:
                break
        top5[b] = picks
    return top5


def _run_v3(exe, bank_sh, t, bank):
    """Index-free path: per-chunk top-8 values only (exact fp32, a
    deterministic superset of the per-chunk top-5); the host recovers
    indices by recomputing the <=8 best 500-wide chunks per row."""
    global LAST_RESULTS
    np_dt = _np_dtype(DTYPE)
    tT_c = np.ascontiguousarray(t.T).astype(np_dt)
    concat = {
        "bankT": bank_sh,
        "tT": np.concatenate([tT_c] * N_CORES, axis=0),
    }
    results = exe([concat[n] for n in exe.in_names])
    LAST_RESULTS = results

    n_grp = KL // KT                                 # 32 chunks of 500
    vals = np.stack([r["cand_v"] for r in results], axis=1)
    vals = vals.reshape(B, -1)                       # [B, 8*32*8=2048]
    # candidate slot -> global chunk start (chunk known from position)
    cores = np.repeat(np.arange(N_CORES, dtype=np.int64), 8 * n_grp)
    chunks = np.tile(np.repeat(np.arange(n_grp, dtype=np.int64), 8), N_CORES)
    wstart = (cores * KL + chunks * KT)[None, :]     # [1, 2048]

    # every true top-5 element is a candidate with a top-5 value, so the
    # top-8 candidate windows per row cover them deterministically
    order = np.argsort(-vals, axis=1, kind="stable")[:, :8]
    sel = np.take_along_axis(np.broadcast_to(wstart, vals.shape),
                             order, axis=1)          # [B, 8]

    bf = ml_dtypes.bfloat16
    bank_bf = bank.astype(bf).astype(np.float32)     # [K, C]
    t_bf = t.astype(bf).astype(np.float32)           # [B, C]
    top5 = np.empty((B, TOPK), np.int64)
    span = np.arange(KT, dtype=np.int64)
    for b in range(B):
        starts = np.unique(sel[b])
        widx = (starts[:, None] + span[None, :]).reshape(-1)
        wsims = bank_bf[widx] @ t_bf[b]              # exact bf16-input sims
        o = np.lexsort((widx, -wsims))
        top5[b] = widx[o[:TOPK]]
    return top5


def kernel(query, current_target, queue, labels, labels_queue):
    query = np.asarray(query, np.float32)
    t = np.asarray(current_target, np.float32)
    queue_f = np.asarray(queue, np.float32)
    labels = np.asarray(labels)
    labels_queue = np.asarray(labels_queue)

    # Host prep: normalize bank rows (fp32, matching reference), transpose.
    norms = np.maximum(np.linalg.norm(queue_f, axis=1), EPS)
    bank = queue_f / norms[:, None]                 # [K, C], normalized
    tT = np.ascontiguousarray(t.T)                  # [C, B]

    np_dt = _np_dtype(DTYPE)
    exe = _get_exec()
    if MODE == "v10":
        top5 = _run_v10(exe, t, bank)
    elif MODE == "v9":
        top5 = _run_v9(exe, t, bank)
    elif MODE == "v8":
        top5 = _run_v8(exe, t, bank)
    elif MODE == "v7":
        top5 = _run_v7(exe, t, bank)
    elif MODE == "v6":
        top5 = _run_v6(exe, t, bank)
    elif MODE == "v5":
        top5 = _run_v5(exe, t, bank)
    elif MODE == "v4":
        top5 = _run_v4(exe, t, bank)
    else:
        # [8*C, KL]: core m's shard (rows m*C..(m+1)*C) is
        # bank[m*KL:(m+1)*KL].T
        bank_sh = np.ascontiguousarray(
            bank.reshape(N_CORES, KL, C).transpose(0, 2, 1)
        ).astype(np_dt).reshape(N_CORES * C, KL)
        if MODE == "v2":
            top5 = _run_v2(exe, bank_sh, t, bank)
        elif MODE == "v3":
            top5 = _run_v3(exe, bank_sh, t, bank)
        else:
            top5 = _run_v1(exe, bank_sh, t, tT)

    # dist_q at the selected indices + purity.
    q_norm = query / np.maximum(
        np.linalg.norm(query, axis=1, keepdims=True), EPS
    )
    rows = bank[top5.reshape(-1)].reshape(B, TOPK, C)          # normalized
    nn_dist_q = 2.0 - 2.0 * np.einsum(
        "bjc,bc->bj", rows.astype(np.float64), q_norm.astype(np.float64)
    )
    loss = nn_dist_q.mean()
    matches = labels_queue[top5] == labels[:, None]
    purity = matches.mean()
    return (np.float32(loss), np.float32(purity))

